# revision 24
# baseline (speedup 1.0000x reference)
"""Trainium2 Bass kernel: masked multi-head attention, sharded across 8 NeuronCores.

Problem shapes (hardcoded): B=2, T=2048, D=1024, H=16 heads, dh=64.

Sharding: one SPMD program with two phases (one per batch element). In each
phase every core handles 2 of the 16 heads (core c -> heads 2c, 2c+1), so the
16 heads of each batch are spread over all 8 cores. This load-balances the
data-dependent work (Q_len/V_len trim the q/k tile counts per batch).

v2 changes vs the fp32 baseline:
  - bf16 inputs/weights/intermediates: matmuls run at 1 cycle/row instead of
    fp32's 4 (fp32 lowers to 2 half-speed passes on TRN2), DMA bytes halve.
  - The two heads' S^T matmuls (K=64 each) are row-tiled to disjoint PE
    quadrants (tile_position (0,0)/(64,0)) so they execute concurrently.
  - exp() for both heads merged into one ACT instruction over a 2-bank PSUM
    tile [128, 2, n] (ACT is the #2 engine; fewer/larger instrs).
  - Epilogue: numerator copied once (DVE), softmax denominator row pulled out
    of PSUM by a tiny DMA, reciprocal_approx_fast on DVE (the old
    single-lane RECIPROCAL was 2.2us/chunk), broadcast over partitions with a
    K=1 f32r matmul, one fused multiply per head.
  - Query-length masking moved to the host gather (rows >= Q_len are simply
    not copied out; the output buffer is pre-zeroed) - no qmask work on HW.
  - The second batch's projections are emitted as filler units inside the
    first batch's ACT-paced attention ladder to keep the PE busy.
"""

import math
import os
from collections import deque
from contextlib import ExitStack

import numpy as np
import ml_dtypes

import concourse.bacc as bacc
import concourse.mybir as mybir
import concourse.tile as tile
from concourse.bass_utils import run_bass_kernel_spmd

F32 = mybir.dt.float32
F32R = mybir.dt.float32r
BF16 = mybir.dt.bfloat16
EXP = mybir.ActivationFunctionType.Exp
BNP = ml_dtypes.bfloat16

B, T, D, H, DH = 2, 2048, 1024, 16, 64
N_CORES = 8
KCH = D // 128          # 8 contraction chunks of the model dim
NEG_BIG = 1.0e12
SCALE = 1.0 / math.sqrt(DH)

LAST_EXEC_NS = None     # filled when BASS_TRACE=1


def _ensure_ntff_hook():
    """run_bass_kernel_spmd(trace=True) imports antenv.axon_hooks, which some
    containers lack; synthesize it (backed by libaxon_pjrt's NRT profiling)
    so tracing degrades gracefully instead of crashing."""
    import sys
    import types
    try:
        import antenv.axon_hooks  # noqa: F401
        return
    except ImportError:
        pass
    try:
        import antenv
        from trn_agent_boot.trn_boot import _ntff_profile_via_ctypes
        hook = _ntff_profile_via_ctypes("/opt/axon/libaxon_pjrt.so")
    except Exception:
        antenv = None
        hook = None
    try:
        m = types.ModuleType("antenv.axon_hooks")
        m._hook = hook
        m.set_axon_ntff_profile_hook = lambda h: setattr(m, "_hook", h)
        m.get_axon_ntff_profile_hook = lambda: m._hook
        sys.modules["antenv.axon_hooks"] = m
        if antenv is not None:
            antenv.axon_hooks = m
    except Exception:
        pass


def _ceil_div(a, b):
    return -(-a // b)


def _chunks(total, w=512):
    out = []
    c = 0
    while c < total:
        out.append((c, min(w, total - c)))
        c += w
    return out


class _Emitter:
    def __init__(self, nc, P, wts):
        self.nc = nc
        self.P = P
        self.wts = wts

    # ---------- projection units (each returns nothing, emits instrs) ------

    def kproj_chunk(self, ph, ci, kr=(0, KCH), st=None):
        """Project keys chunk ci: kc[:, c0:c0+n] = (WK.T @ K_seq.T) slice.
        `kr` bounds the contraction range so a chunk can be emitted as two
        filler halves sharing the psum tile passed via `st`."""
        nc, P = self.nc, self.P
        c0, n = ph["kch"][ci]
        xt = ph["xk_tiles"][ci]
        if kr[0] == 0:
            ps = P["pp"].tile([128, 512], F32, tag="pp", name="kps")
            if st is not None:
                st["ps"] = ps
        else:
            ps = st["ps"]
        for k in range(*kr):
            nc.tensor.matmul(ps[:, :n], lhsT=self.wts["wk"][:, k, :],
                             rhs=xt[:, k, :n],
                             start=(k == 0), stop=(k == KCH - 1),
                             skip_group_check=True)
        if kr[1] == KCH:
            nc.vector.tensor_copy(ph["kc"][:, c0:c0 + n], ps[:, :n])

    def vproj_tile(self, ph, m):
        """Project value tokens [m*128,(m+1)*128) into va[:, m, :, 0:64]."""
        nc, P = self.nc, self.P
        ci, r = divmod(m * 128, 512)
        c0, cn = ph["vch"][ci]
        xt = ph["xv_tiles"][ci]
        ps = P["pp"].tile([128, 512], F32, tag="pp", name="vps")
        for k in range(KCH):
            nc.tensor.matmul(ps[:, 0:128], lhsT=xt[:, k, r:r + 128],
                             rhs=self.wts["wv"][:, k, :],
                             start=(k == 0), stop=(k == KCH - 1),
                             skip_group_check=True)
        nc.vector.tensor_copy(
            ph["va"][:, m, :, 0:64],
            ps[:, 0:128].rearrange("p (g d) -> p g d", g=2))

    def qproj_chunk(self, ph, ci):
        """Project queries chunk ci into the qc ring; returns the tile."""
        nc, P = self.nc, self.P
        c0, n = ph["qch"][ci]
        xt = ph["xq_tiles"][ci]
        ps = P["pp"].tile([128, 512], F32, tag="pp", name="qps")
        for k in range(KCH):
            nc.tensor.matmul(ps[:, :n], lhsT=self.wts["wq"][:, k, :],
                             rhs=xt[:, k, :n],
                             start=(k == 0), stop=(k == KCH - 1),
                             skip_group_check=True)
        qc = P["qc"].tile([128, 512], BF16, tag="qc" + str(ph["b"]),
                          name="qc", bufs=3)
        nc.vector.tensor_copy(qc[:, :n], ps[:, :n])
        ph["qcs"][ci] = qc
        return qc

    # ---------- attention ladder ------------------------------------------

    def ladder(self, ph, ci, due, anytime):
        """S/exp/PV software pipeline for q chunk ci.

        `due`: deque of (deadline_step, closure) in non-decreasing deadline
        order — every unit whose deadline has arrived is emitted that step
        (these carry dataflow deadlines, e.g. vproj(kt) before PV(kt)).
        `anytime`: deque of independent filler closures; at most one is
        popped per step, only on steps with no due unit (keeps PE work per
        step under the ACT exp cadence)."""
        nc, P = self.nc, self.P
        c0, n = ph["qch"][ci]
        NK = ph["NK"]
        qc = ph["qcs"].pop(ci)
        kb = ph["kb_tile"]
        kc, va = ph["kc"], ph["va"]
        scale = ph["scale"]

        otd = P["ot"].tile([65, 2, 512], F32, tag="ot", name="otd")

        def emit_s(kt):
            sps = P["sp"].tile([128, 2, 512], F32, tag="sp", name="sps")
            for h in (0, 1):
                nc.tensor.matmul(
                    sps[:, h, :n],
                    lhsT=kc[h * 64:(h + 1) * 64, kt * 128:(kt + 1) * 128],
                    rhs=qc[h * 64:(h + 1) * 64, :n],
                    start=True, stop=True,
                    tile_position=(h * 64, 0),
                    skip_group_check=True)
            e = P["e"].tile([128, 2, 512], BF16, tag="e", name="e", bufs=3)
            nc.scalar.activation(e[:, :, :n], sps[:, :, :n], EXP,
                                 bias=kb[:, kt:kt + 1], scale=scale)
            return e

        ep = emit_s(0)
        for kt in range(NK):
            ec = ep
            if kt + 1 < NK:
                ep = emit_s(kt + 1)
            popped = False
            while due and due[0][0] <= kt:
                due.popleft()[1]()
                popped = True
            if not popped and anytime:
                anytime.popleft()()
            for h in (0, 1):
                nc.tensor.matmul(otd[:, h, :n], lhsT=va[:, kt, h, :],
                                 rhs=ec[:, h, :n],
                                 start=(kt == 0), stop=(kt == NK - 1),
                                 skip_group_check=True)
        return otd

    def epilogue(self, ph, ci, otd):
        """Normalize otd -> OTs[:, :, c0:c0+n] (no qmask: host trims)."""
        nc, P = self.nc, self.P
        c0, n = ph["qch"][ci]
        ou = P["ou"].tile([64, 2, 512], BF16, tag="ou", name="ou", bufs=2)
        nc.vector.tensor_copy(ou[:, :, :n], otd[0:64, :, :n])
        drow = P["rows"].tile([65, 2, 512], BF16, tag="drow", name="drow",
                              bufs=2)
        nc.vector.tensor_copy(drow[64:65, :, :n], otd[64:65, :, :n])
        rsb = P["rows"].tile([64, 2, 512], F32, tag="rsb", name="rsb",
                             bufs=2)
        for h in (0, 1):
            # broadcast d over 64 partitions (K=1 bf16 matmul), then
            # reciprocal on the [64, n] block (DVE cost is free-size-based,
            # so this is no dearer than a single-partition reciprocal).
            dps = P["pp"].tile([128, 512], F32, tag="pp", name="dps")
            nc.tensor.matmul(dps[0:64, :n],
                             lhsT=P["onesr"][64:65, 0:64],
                             rhs=drow[64:65, h, :n],
                             start=True, stop=True, skip_group_check=True)
            nc.vector.reciprocal_approx_fast(rsb[:, h, :n], dps[0:64, :n])
            nc.vector.tensor_mul(ph["OTs"][:, h, c0:c0 + n],
                                 ou[:, h, :n], rsb[:, h, :n])


def _chunk0_due(em, ph):
    """Deadline units for the first q chunk's ladder: the phase's remaining
    k-proj chunks and all v-proj tiles, interleaved in the exact order their
    DMA chunks arrive, plus qproj(1). Deadlines: vproj(m) before PV(m)
    (popped a step early so the DVE copy hides), kproj(ci) before S(4ci)
    which is emitted at step 4ci-1, qproj(1) a few steps before chunk end."""
    NK = ph["NK"]
    due = []
    for m in range(NK):
        if m >= 1 and m % 4 == 0:
            due.append((m - 2, lambda ci=m // 4: em.kproj_chunk(ph, ci)))
        due.append((max(0, m - 1), lambda m=m: em.vproj_tile(ph, m)))
    if len(ph["qch"]) > 1:
        d = max(0, NK - 3)
        pos = next((i for i, u in enumerate(due) if u[0] > d), len(due))
        due.insert(pos, (d, lambda: em.qproj_chunk(ph, 1)))
    return deque(due)


def _phase_units(em, ph):
    """Independent filler closures projecting all of phase `ph`'s inputs,
    in DMA-arrival order. kproj chunks are split in two halves so a single
    pop stays under the ladder's per-step PE budget."""
    units = []
    for ci in range(len(ph["kch"])):
        st = {}
        units.append(lambda ci=ci, st=st:
                     em.kproj_chunk(ph, ci, kr=(0, KCH // 2), st=st))
        units.append(lambda ci=ci, st=st:
                     em.kproj_chunk(ph, ci, kr=(KCH // 2, KCH), st=st))
        for m in range(ci * 4, min((ci + 1) * 4, ph["NK"])):
            units.append(lambda m=m: em.vproj_tile(ph, m))
    units.append(lambda: em.qproj_chunk(ph, 0))
    return units


def _build_program(phases):
    nc = bacc.Bacc("TRN2", target_bir_lowering=False, debug=False,
                   num_devices=N_CORES)
    for ph in phases:
        s = str(ph["b"])
        Qp, Kp, NK = ph["Qp"], ph["Kp"], ph["NK"]
        ph["qch"] = _chunks(Qp)
        ph["kch"] = _chunks(Kp)
        ph["vch"] = ph["kch"]
        ph["qcs"] = {}
        io = {
            "kb": nc.dram_tensor("kb" + s, [128, NK], F32, kind="ExternalInput"),
            "out": nc.dram_tensor("out" + s, [64, 2, Qp], BF16, kind="ExternalOutput"),
        }
        # per-chunk input tensors: per-partition-contiguous so each DMA
        # lowers to 128 large descriptors instead of 1KB-strided fragments
        for key, chl in (("xq", ph["qch"]), ("xk", ph["kch"]), ("xv", ph["vch"])):
            for ci, (c0, n) in enumerate(chl):
                io[f"{key}c{ci}"] = nc.dram_tensor(
                    f"{key}{s}c{ci}", [128, KCH, n], BF16, kind="ExternalInput")
        ph["io"] = io

    with tile.TileContext(nc) as tc, ExitStack() as ctx:
        P = {
            "w": ctx.enter_context(tc.tile_pool(name="w", bufs=1)),
            "x": ctx.enter_context(tc.tile_pool(name="x", bufs=1)),
            "qc": ctx.enter_context(tc.tile_pool(name="qc", bufs=3)),
            "e": ctx.enter_context(tc.tile_pool(name="e", bufs=3)),
            "ou": ctx.enter_context(tc.tile_pool(name="ou", bufs=2)),
            "rows": ctx.enter_context(tc.tile_pool(name="rows", bufs=2)),
            "persist": ctx.enter_context(tc.tile_pool(name="persist", bufs=1)),
            "pp": ctx.enter_context(tc.tile_pool(name="pp", bufs=2, space="PSUM")),
            "sp": ctx.enter_context(tc.tile_pool(name="sp", bufs=2, space="PSUM")),
            "ot": ctx.enter_context(tc.tile_pool(name="ot", bufs=1, space="PSUM")),
        }
        onesr = P["w"].tile([65, 64], BF16, tag="onesr", name="onesr")
        nc.vector.memset(onesr[64:65, :], 1.0)
        P["onesr"] = onesr
        warm = P["w"].tile([1, 1], F32, tag="actwarm", name="actwarm")
        nc.vector.memset(warm[:], 0.0)
        nc.scalar.activation(warm[:], warm[:], EXP)

        # PE p-state warmup: dummy bf16 matmuls on zeroed tiles keep the PE
        # clocking up while the first input DMAs land.
        zw = P["w"].tile([128, 128], BF16, tag="zw", name="zw")
        nc.gpsimd.memset(zw[:], 0.0)
        zw2 = P["w"].tile([128, 512], BF16, tag="zw2", name="zw2")
        nc.gpsimd.memset(zw2[:], 0.0)
        for _ in range(2):
            wps = P["sp"].tile([128, 2, 512], F32, tag="sp", name="wps")
            for r in range(4):
                nc.tensor.matmul(wps[:, 0, :], lhsT=zw[:], rhs=zw2[:],
                                 start=(r == 0), stop=(r == 3),
                                 skip_group_check=True)

        # -------- weights --------
        wts = {}
        for nm, eng in (("wk", nc.scalar), ("wq", nc.scalar), ("wv", nc.sync)):
            wd = nc.dram_tensor(nm, [128, KCH, 128], BF16, kind="ExternalInput")
            t = P["w"].tile([128, KCH, 128], BF16, tag=nm, name=nm)
            eng.dma_start(t[:], wd[:])
            wts[nm] = t

        # -------- input staging (issue order == consumption order) --------
        A = phases[0]
        Bp = phases[1] if len(phases) > 1 else None
        for ph in phases:
            s = str(ph["b"])
            kb = P["w"].tile([128, ph["NK"]], F32, tag="kb" + s, name="kb")
            nc.sync.dma_start(kb[:], ph["io"]["kb"][:])
            ph["kb_tile"] = kb
            for key, chl in (("xq", ph["qch"]), ("xk", ph["kch"]),
                             ("xv", ph["vch"])):
                ph[f"{key}_tiles"] = [None] * len(chl)

        def stage1(ph, key, ci, eng, halves=1):
            """One input chunk -> SBUF, issued from `eng` (DMA trigger issue
            is ~0.6us+size serial per issuing sequencer, so spread engines)."""
            s = str(ph["b"])
            n = dict(xq=ph["qch"], xk=ph["kch"], xv=ph["vch"])[key][ci][1]
            xt = P["x"].tile([128, KCH, n], BF16, tag=f"{key}{s}c{ci}",
                             name=f"{key}{s}c{ci}", bufs=1)
            src = ph["io"][f"{key}c{ci}"]
            step = KCH // halves
            for k in range(0, KCH, step):
                eng.dma_start(xt[:, k:k + step, :], src[:, k:k + step, :])
            ph[f"{key}_tiles"][ci] = xt

        # ACT issues the head-critical chunks (it is idle until they land),
        # SP the rest of phase A, GpSimd (software DGE) the tail + phase B.
        nkA, nqA = len(A["kch"]), len(A["qch"])
        stage1(A, "xk", 0, nc.scalar, halves=2)
        stage1(A, "xq", 0, nc.scalar)
        stage1(A, "xv", 0, nc.sync)
        for ci in range(1, nkA):
            stage1(A, "xk", ci, nc.sync)
            if ci == nkA - 1 and nqA > 1:
                stage1(A, "xq", 1, nc.scalar)
            stage1(A, "xv", ci, nc.sync)
        if nkA == 1 and nqA > 1:
            stage1(A, "xq", 1, nc.scalar)
        for ci in range(2, nqA):
            stage1(A, "xq", ci, nc.gpsimd)
        if Bp is not None:
            for ci in range(len(Bp["kch"])):
                stage1(Bp, "xk", ci, nc.gpsimd)
                stage1(Bp, "xv", ci, nc.gpsimd)
            for ci in range(len(Bp["qch"])):
                stage1(Bp, "xq", ci, nc.gpsimd)

        # -------- persistent per-phase tiles --------
        for ph in phases:
            s = str(ph["b"])
            ph["kc"] = P["persist"].tile([128, ph["Kp"]], BF16,
                                         tag="kc" + s, name="kc" + s)
            ph["va"] = P["persist"].tile([128, ph["NK"], 2, 65], BF16,
                                         tag="va" + s, name="va" + s)
            nc.gpsimd.memset(ph["va"][:, :, :, 64:65], 1.0)
            ph["OTs"] = P["persist"].tile([64, 2, ph["Qp"]], BF16,
                                          tag="oT" + s, name="oT" + s)

        em = _Emitter(nc, P, wts)

        # -------- phase A flow --------
        em.kproj_chunk(A, 0)
        em.qproj_chunk(A, 0)
        rest = deque(_phase_units(em, Bp)) if Bp is not None else deque()
        for ci in range(nqA):
            if ci == 0:
                due = _chunk0_due(em, A)
                anytime = deque()
            else:
                due = deque()
                if ci + 1 < nqA:
                    due.append((max(0, A["NK"] - 3),
                                lambda ci=ci: em.qproj_chunk(A, ci + 1)))
                # B's fillers from chunk 2 on (their DMA lands after A's)
                anytime = rest if ci >= 2 else deque()
            otd = em.ladder(A, ci, due, anytime)
            em.epilogue(A, ci, otd)
            c0, n = A["qch"][ci]
            nc.sync.dma_start(A["io"]["out"][:, :, c0:c0 + n],
                              A["OTs"][:, :, c0:c0 + n])

        # -------- phase B flow --------
        if Bp is not None:
            while rest:
                rest.popleft()()
            if 0 not in Bp["qcs"]:
                em.qproj_chunk(Bp, 0)
            for ci in range(len(Bp["qch"])):
                otd = em.ladder(Bp, ci, deque(), deque())
                if ci + 1 < len(Bp["qch"]):
                    em.qproj_chunk(Bp, ci + 1)
                em.epilogue(Bp, ci, otd)
                c0, n = Bp["qch"][ci]
                nc.sync.dma_start(Bp["io"]["out"][:, :, c0:c0 + n],
                                  Bp["OTs"][:, :, c0:c0 + n])

    nc.compile()
    return nc


def _prep_xT(X, Pq):
    """[T, D] -> [128, KCH, Pq] bf16 with x[p, k, t] = X[t, k*128 + p]."""
    Xp = np.ascontiguousarray(X[:Pq].T)                 # [D, Pq]
    return np.ascontiguousarray(
        Xp.reshape(KCH, 128, Pq).transpose(1, 0, 2)).astype(BNP)


def _prep_w(W, c):
    """[D, H*DH] -> per-core [128, KCH, 128] bf16 slice of heads (2c, 2c+1)."""
    Ws = W[:, c * 128:(c + 1) * 128]                    # [D, 128]
    return np.ascontiguousarray(
        Ws.reshape(KCH, 128, 128).transpose(1, 0, 2)).astype(BNP)


def kernel(Q_seq, K_seq, V_seq, Q_len, V_len, WQ, WK, WV):
    global LAST_EXEC_NS
    Q_seq = np.asarray(Q_seq, dtype=np.float32)
    K_seq = np.asarray(K_seq, dtype=np.float32)
    V_seq = np.asarray(V_seq, dtype=np.float32)
    WQ = np.asarray(WQ, dtype=np.float32)
    WK = np.asarray(WK, dtype=np.float32)
    WV = np.asarray(WV, dtype=np.float32)
    qlen = [int(np.asarray(Q_len)[b, 0]) for b in range(B)]
    vlen = [int(np.asarray(V_len)[b, 0]) for b in range(B)]

    phases = []
    for b in range(B):
        Qp = _ceil_div(qlen[b], 32) * 32   # q only needs 32-elem alignment
        if Qp == 0:
            continue  # whole batch output is zero
        if vlen[b] > 0:
            NK, scale = _ceil_div(vlen[b], 128), SCALE
        else:
            # all keys masked -> reference softmax degenerates to uniform
            # over all T keys; exp(0*S + 0) = 1 reproduces it exactly.
            NK, scale = T // 128, 0.0
        phases.append(dict(b=b, NK=NK, Qp=Qp, Kp=NK * 128, scale=scale))
    phases.sort(key=lambda ph: -ph["Qp"])  # big phase first (filler donor)

    out = np.zeros((B, T, H * DH), dtype=np.float32)
    if not phases:
        return out

    nc = _build_program(phases)

    # per-phase data shared by all cores
    shared = {}
    for ph in phases:
        b, s, Qp, Kp, NK = ph["b"], str(ph["b"]), ph["Qp"], ph["Kp"], ph["NK"]
        kbias = np.where(np.arange(Kp) < vlen[b], 0.0,
                         -NEG_BIG if vlen[b] > 0 else 0.0)
        kbias = np.ascontiguousarray(
            kbias.astype(np.float32).reshape(NK, 128).T)        # [128, NK]
        d = {"kb" + s: kbias}
        for key, X, Pq in (("xq", Q_seq[b], Qp), ("xk", K_seq[b], Kp),
                           ("xv", V_seq[b], Kp)):
            full = _prep_xT(X, Pq)                              # [128, KCH, Pq]
            for ci, (c0, n) in enumerate(_chunks(Pq)):
                d[f"{key}{s}c{ci}"] = np.ascontiguousarray(
                    full[:, :, c0:c0 + n])
        shared[s] = d

    in_maps = []
    for c in range(N_CORES):
        m = {}
        for ph in phases:
            m.update(shared[str(ph["b"])])
        m["wq"] = _prep_w(WQ, c)
        m["wk"] = _prep_w(WK, c)
        m["wv"] = _prep_w(WV, c)
        in_maps.append(m)

    trace = bool(os.environ.get("BASS_TRACE"))
    if trace:
        _ensure_ntff_hook()
    res = run_bass_kernel_spmd(nc, in_maps, list(range(N_CORES)), trace=trace)
    LAST_EXEC_NS = res.exec_time_ns

    for c in range(N_CORES):
        r = res.results[c]
        for ph in phases:
            b, s, ql = ph["b"], str(ph["b"]), qlen[ph["b"]]
            o = np.asarray(r["out" + s]).astype(np.float32)  # [64, 2, Qp]
            for h in (0, 1):
                head = 2 * c + h
                out[b, :ql, head * DH:(head + 1) * DH] = o[:, h, :ql].T
    return out


# revision 27
# speedup vs baseline: 1.2478x; 1.2478x over previous
"""Trainium2 Bass kernel: masked multi-head attention, sharded across 8 NeuronCores.

Problem shapes (hardcoded): B=2, T=2048, D=1024, H=16 heads, dh=64.

Sharding: one SPMD program with two phases (one per batch element). In each
phase every core handles 2 of the 16 heads (core c -> heads 2c, 2c+1), so the
16 heads of each batch are spread over all 8 cores. This load-balances the
data-dependent work (Q_len/V_len trim the q/k tile counts per batch).

v2 changes vs the fp32 baseline:
  - bf16 inputs/weights/intermediates: matmuls run at 1 cycle/row instead of
    fp32's 4 (fp32 lowers to 2 half-speed passes on TRN2), DMA bytes halve.
  - The two heads' S^T matmuls (K=64 each) are row-tiled to disjoint PE
    quadrants (tile_position (0,0)/(64,0)) so they execute concurrently.
  - exp() for both heads merged into one ACT instruction over a 2-bank PSUM
    tile [128, 2, n] (ACT is the #2 engine; fewer/larger instrs).
  - Epilogue: numerator copied once (DVE), softmax denominator row pulled out
    of PSUM by a tiny DMA, reciprocal_approx_fast on DVE (the old
    single-lane RECIPROCAL was 2.2us/chunk), broadcast over partitions with a
    K=1 f32r matmul, one fused multiply per head.
  - Query-length masking moved to the host gather (rows >= Q_len are simply
    not copied out; the output buffer is pre-zeroed) - no qmask work on HW.
  - The second batch's projections are emitted as filler units inside the
    first batch's ACT-paced attention ladder to keep the PE busy.
"""

import math
import os
from collections import deque
from contextlib import ExitStack

import numpy as np
import ml_dtypes

import concourse.bacc as bacc
import concourse.mybir as mybir
import concourse.tile as tile
from concourse.bass_utils import run_bass_kernel_spmd

F32 = mybir.dt.float32
F32R = mybir.dt.float32r
BF16 = mybir.dt.bfloat16
EXP = mybir.ActivationFunctionType.Exp
BNP = ml_dtypes.bfloat16

B, T, D, H, DH = 2, 2048, 1024, 16, 64
N_CORES = 8
KCH = D // 128          # 8 contraction chunks of the model dim
NEG_BIG = 1.0e12
SCALE = 1.0 / math.sqrt(DH)

LAST_EXEC_NS = None     # filled when BASS_TRACE=1


def _ensure_ntff_hook():
    """run_bass_kernel_spmd(trace=True) imports antenv.axon_hooks, which some
    containers lack; synthesize it (backed by libaxon_pjrt's NRT profiling)
    so tracing degrades gracefully instead of crashing."""
    import sys
    import types
    try:
        import antenv.axon_hooks  # noqa: F401
        return
    except ImportError:
        pass
    try:
        import antenv
        from trn_agent_boot.trn_boot import _ntff_profile_via_ctypes
        hook = _ntff_profile_via_ctypes("/opt/axon/libaxon_pjrt.so")
    except Exception:
        antenv = None
        hook = None
    try:
        m = types.ModuleType("antenv.axon_hooks")
        m._hook = hook
        m.set_axon_ntff_profile_hook = lambda h: setattr(m, "_hook", h)
        m.get_axon_ntff_profile_hook = lambda: m._hook
        sys.modules["antenv.axon_hooks"] = m
        if antenv is not None:
            antenv.axon_hooks = m
    except Exception:
        pass


def _ceil_div(a, b):
    return -(-a // b)


def _chunks(total, w=512):
    out = []
    c = 0
    while c < total:
        out.append((c, min(w, total - c)))
        c += w
    return out


class _Emitter:
    def __init__(self, nc, P, wts):
        self.nc = nc
        self.P = P
        self.wts = wts

    # ---------- projection units (each returns nothing, emits instrs) ------

    def kproj_chunk(self, ph, ci, kr=(0, KCH), st=None):
        """Project keys chunk ci: kc[:, c0:c0+n] = (WK.T @ K_seq.T) slice.
        `kr` bounds the contraction range so a chunk can be emitted as two
        filler halves sharing the psum tile passed via `st`."""
        nc, P = self.nc, self.P
        c0, n = ph["kch"][ci]
        xt = ph["xk_tiles"][ci]
        if kr[0] == 0:
            ps = P["pp"].tile([128, 512], F32, tag="pp", name="kps")
            if st is not None:
                st["ps"] = ps
        else:
            ps = st["ps"]
        for k in range(*kr):
            nc.tensor.matmul(ps[:, :n], lhsT=self.wts["wk"][:, k, :],
                             rhs=xt[:, k, :n],
                             start=(k == 0), stop=(k == KCH - 1),
                             skip_group_check=True)
        if kr[1] == KCH:
            nc.vector.tensor_copy(ph["kc"][:, c0:c0 + n], ps[:, :n])

    def vproj_tile(self, ph, m):
        """Project value tokens [m*128,(m+1)*128) into va[:, m, :, 0:64]."""
        nc, P = self.nc, self.P
        ci, r = divmod(m * 128, 512)
        c0, cn = ph["vch"][ci]
        xt = ph["xv_tiles"][ci]
        ps = P["pp"].tile([128, 512], F32, tag="pp", name="vps")
        for k in range(KCH):
            nc.tensor.matmul(ps[:, 0:128], lhsT=xt[:, k, r:r + 128],
                             rhs=self.wts["wv"][:, k, :],
                             start=(k == 0), stop=(k == KCH - 1),
                             skip_group_check=True)
        nc.vector.tensor_copy(
            ph["va"][:, m, :, 0:64],
            ps[:, 0:128].rearrange("p (g d) -> p g d", g=2))

    def qproj_chunk(self, ph, ci):
        """Project queries chunk ci into the qc ring; returns the tile."""
        nc, P = self.nc, self.P
        c0, n = ph["qch"][ci]
        xt = ph["xq_tiles"][ci]
        ps = P["pp"].tile([128, 512], F32, tag="pp", name="qps")
        for k in range(KCH):
            nc.tensor.matmul(ps[:, :n], lhsT=self.wts["wq"][:, k, :],
                             rhs=xt[:, k, :n],
                             start=(k == 0), stop=(k == KCH - 1),
                             skip_group_check=True)
        qc = P["qc"].tile([128, 512], BF16, tag="qc" + str(ph["b"]),
                          name="qc", bufs=3)
        nc.vector.tensor_copy(qc[:, :n], ps[:, :n])
        ph["qcs"][ci] = qc
        return qc

    # ---------- attention ladder ------------------------------------------

    def ladder(self, ph, ci, due, anytime):
        """S/exp/PV software pipeline for q chunk ci.

        `due`: deque of (deadline_step, closure) in non-decreasing deadline
        order — every unit whose deadline has arrived is emitted that step
        (these carry dataflow deadlines, e.g. vproj(kt) before PV(kt)).
        `anytime`: deque of independent filler closures; at most one is
        popped per step, only on steps with no due unit (keeps PE work per
        step under the ACT exp cadence)."""
        nc, P = self.nc, self.P
        c0, n = ph["qch"][ci]
        NK = ph["NK"]
        qc = ph["qcs"].pop(ci)
        kb = ph["kb_tile"]
        kc, va = ph["kc"], ph["va"]
        scale = ph["scale"]

        otd = P["ot"].tile([65, 2, 512], F32, tag="ot", name="otd")

        def emit_s(kt):
            sps = P["sp"].tile([128, 2, 512], F32, tag="sp", name="sps")
            for h in (0, 1):
                nc.tensor.matmul(
                    sps[:, h, :n],
                    lhsT=kc[h * 64:(h + 1) * 64, kt * 128:(kt + 1) * 128],
                    rhs=qc[h * 64:(h + 1) * 64, :n],
                    start=True, stop=True,
                    tile_position=(h * 64, 0),
                    skip_group_check=True)
            e = P["e"].tile([128, 2, 512], BF16, tag="e", name="e", bufs=3)
            nc.scalar.activation(e[:, :, :n], sps[:, :, :n], EXP,
                                 bias=kb[:, kt:kt + 1], scale=scale)
            return e

        ep = emit_s(0)
        for kt in range(NK):
            ec = ep
            if kt + 1 < NK:
                ep = emit_s(kt + 1)
            popped = False
            while due and due[0][0] <= kt:
                due.popleft()[1]()
                popped = True
            if not popped and anytime:
                anytime.popleft()()
            for h in (0, 1):
                nc.tensor.matmul(otd[:, h, :n], lhsT=va[:, kt, h, :],
                                 rhs=ec[:, h, :n],
                                 start=(kt == 0), stop=(kt == NK - 1),
                                 skip_group_check=True)
        return otd

    def epilogue(self, ph, ci, otd):
        """Normalize otd -> OTs[:, :, c0:c0+n] (no qmask: host trims)."""
        nc, P = self.nc, self.P
        c0, n = ph["qch"][ci]
        ou = P["ou"].tile([64, 2, 512], BF16, tag="ou", name="ou", bufs=2)
        nc.vector.tensor_copy(ou[:, :, :n], otd[0:64, :, :n])
        drow = P["rows"].tile([65, 2, 512], BF16, tag="drow", name="drow",
                              bufs=2)
        nc.vector.tensor_copy(drow[64:65, :, :n], otd[64:65, :, :n])
        rsb = P["rows"].tile([64, 2, 512], F32, tag="rsb", name="rsb",
                             bufs=2)
        for h in (0, 1):
            # broadcast d over 64 partitions (K=1 bf16 matmul), then
            # reciprocal on the [64, n] block (DVE cost is free-size-based,
            # so this is no dearer than a single-partition reciprocal).
            dps = P["pp"].tile([128, 512], F32, tag="pp", name="dps")
            nc.tensor.matmul(dps[0:64, :n],
                             lhsT=P["onesr"][64:65, 0:64],
                             rhs=drow[64:65, h, :n],
                             start=True, stop=True, skip_group_check=True)
            nc.vector.reciprocal_approx_fast(rsb[:, h, :n], dps[0:64, :n])
            nc.vector.tensor_mul(ph["OTs"][:, h, c0:c0 + n],
                                 ou[:, h, :n], rsb[:, h, :n])


def _chunk0_due(em, ph):
    """Deadline units for the first q chunk's ladder: the phase's remaining
    k-proj chunks and all v-proj tiles, interleaved in the exact order their
    DMA chunks arrive, plus qproj(1). Deadlines: vproj(m) before PV(m)
    (popped a step early so the DVE copy hides), kproj(ci) before S(4ci)
    which is emitted at step 4ci-1, qproj(1) a few steps before chunk end."""
    NK = ph["NK"]
    due = []
    for m in range(NK):
        if m >= 1 and m % 4 == 0:
            due.append((m - 2, lambda ci=m // 4: em.kproj_chunk(ph, ci)))
        due.append((max(0, m - 1), lambda m=m: em.vproj_tile(ph, m)))
    if len(ph["qch"]) > 1:
        d = max(0, NK - 3)
        pos = next((i for i, u in enumerate(due) if u[0] > d), len(due))
        due.insert(pos, (d, lambda: em.qproj_chunk(ph, 1)))
    return deque(due)


def _phase_units(em, ph):
    """Independent filler closures projecting all of phase `ph`'s inputs,
    in DMA-arrival order. kproj chunks are split in two halves so a single
    pop stays under the ladder's per-step PE budget."""
    units = []
    for ci in range(len(ph["kch"])):
        st = {}
        units.append(lambda ci=ci, st=st:
                     em.kproj_chunk(ph, ci, kr=(0, KCH // 2), st=st))
        units.append(lambda ci=ci, st=st:
                     em.kproj_chunk(ph, ci, kr=(KCH // 2, KCH), st=st))
        for m in range(ci * 4, min((ci + 1) * 4, ph["NK"])):
            units.append(lambda m=m: em.vproj_tile(ph, m))
    units.append(lambda: em.qproj_chunk(ph, 0))
    return units


def _build_program(phases):
    nc = bacc.Bacc("TRN2", target_bir_lowering=False, debug=False,
                   num_devices=N_CORES)
    for ph in phases:
        s = str(ph["b"])
        Qp, Kp, NK = ph["Qp"], ph["Kp"], ph["NK"]
        ph["qch"] = _chunks(Qp)
        ph["kch"] = _chunks(Kp)
        ph["vch"] = ph["kch"]
        ph["qcs"] = {}
        io = {
            "kb": nc.dram_tensor("kb" + s, [128, NK], F32, kind="ExternalInput"),
            "out": nc.dram_tensor("out" + s, [64, 2, Qp], BF16, kind="ExternalOutput"),
        }
        # per-chunk input tensors: per-partition-contiguous so each DMA
        # lowers to 128 large descriptors instead of 1KB-strided fragments
        for key, chl in (("xq", ph["qch"]), ("xk", ph["kch"]), ("xv", ph["vch"])):
            for ci, (c0, n) in enumerate(chl):
                io[f"{key}c{ci}"] = nc.dram_tensor(
                    f"{key}{s}c{ci}", [128, KCH, n], BF16, kind="ExternalInput")
        ph["io"] = io

    with tile.TileContext(nc) as tc, ExitStack() as ctx:
        P = {
            "w": ctx.enter_context(tc.tile_pool(name="w", bufs=1)),
            "x": ctx.enter_context(tc.tile_pool(name="x", bufs=1)),
            "qc": ctx.enter_context(tc.tile_pool(name="qc", bufs=3)),
            "e": ctx.enter_context(tc.tile_pool(name="e", bufs=3)),
            "ou": ctx.enter_context(tc.tile_pool(name="ou", bufs=2)),
            "rows": ctx.enter_context(tc.tile_pool(name="rows", bufs=2)),
            "persist": ctx.enter_context(tc.tile_pool(name="persist", bufs=1)),
            "pp": ctx.enter_context(tc.tile_pool(name="pp", bufs=2, space="PSUM")),
            "sp": ctx.enter_context(tc.tile_pool(name="sp", bufs=2, space="PSUM")),
            "ot": ctx.enter_context(tc.tile_pool(name="ot", bufs=1, space="PSUM")),
        }
        onesr = P["w"].tile([65, 64], BF16, tag="onesr", name="onesr")
        nc.vector.memset(onesr[64:65, :], 1.0)
        P["onesr"] = onesr
        warm = P["w"].tile([1, 1], F32, tag="actwarm", name="actwarm")
        nc.vector.memset(warm[:], 0.0)
        nc.scalar.activation(warm[:], warm[:], EXP)

        # PE p-state warmup: dummy bf16 matmuls on zeroed tiles keep the PE
        # clocking up while the first input DMAs land.
        zw = P["w"].tile([128, 128], BF16, tag="zw", name="zw")
        nc.gpsimd.memset(zw[:], 0.0)
        zw2 = P["w"].tile([128, 512], BF16, tag="zw2", name="zw2")
        nc.gpsimd.memset(zw2[:], 0.0)
        for _ in range(2):
            wps = P["sp"].tile([128, 2, 512], F32, tag="sp", name="wps")
            for r in range(4):
                nc.tensor.matmul(wps[:, 0, :], lhsT=zw[:], rhs=zw2[:],
                                 start=(r == 0), stop=(r == 3),
                                 skip_group_check=True)

        # -------- weights --------
        wts = {}
        for nm, eng in (("wk", nc.scalar), ("wq", nc.scalar), ("wv", nc.sync)):
            wd = nc.dram_tensor(nm, [128, KCH, 128], BF16, kind="ExternalInput")
            t = P["w"].tile([128, KCH, 128], BF16, tag=nm, name=nm)
            eng.dma_start(t[:], wd[:])
            wts[nm] = t

        # -------- input staging (issue order == consumption order) --------
        A = phases[0]
        Bp = phases[1] if len(phases) > 1 else None
        for ph in phases:
            s = str(ph["b"])
            kb = P["w"].tile([128, ph["NK"]], F32, tag="kb" + s, name="kb")
            nc.sync.dma_start(kb[:], ph["io"]["kb"][:])
            ph["kb_tile"] = kb
            for key, chl in (("xq", ph["qch"]), ("xk", ph["kch"]),
                             ("xv", ph["vch"])):
                ph[f"{key}_tiles"] = [None] * len(chl)

        def stage1(ph, key, ci, eng, halves=1):
            """One input chunk -> SBUF, issued from `eng` (DMA trigger issue
            is ~0.6us+size serial per issuing sequencer, so spread engines)."""
            s = str(ph["b"])
            n = dict(xq=ph["qch"], xk=ph["kch"], xv=ph["vch"])[key][ci][1]
            xt = P["x"].tile([128, KCH, n], BF16, tag=f"{key}{s}c{ci}",
                             name=f"{key}{s}c{ci}", bufs=1)
            src = ph["io"][f"{key}c{ci}"]
            step = KCH // halves
            for k in range(0, KCH, step):
                eng.dma_start(xt[:, k:k + step, :], src[:, k:k + step, :])
            ph[f"{key}_tiles"][ci] = xt

        # -------- persistent per-phase tiles --------
        for ph in phases:
            s = str(ph["b"])
            ph["kc"] = P["persist"].tile([128, ph["Kp"]], BF16,
                                         tag="kc" + s, name="kc" + s)
            ph["va"] = P["persist"].tile([128, ph["NK"], 2, 65], BF16,
                                         tag="va" + s, name="va" + s)
            nc.gpsimd.memset(ph["va"][:, :, :, 64:65], 1.0)
            ph["OTs"] = P["persist"].tile([64, 2, ph["Qp"]], BF16,
                                          tag="oT" + s, name="oT" + s)

        # ACT issues only the head-critical chunks (it's idle until they
        # land); SP streams the rest of phase A in consumption order.
        # Phase B goes on GpSimd software-DGE, gated behind a mid-phase-A
        # dependency so its transfers can't contend with A's head.
        nkA, nqA = len(A["kch"]), len(A["qch"])
        stage1(A, "xk", 0, nc.scalar, halves=2)
        stage1(A, "xq", 0, nc.sync)
        stage1(A, "xv", 0, nc.sync)
        for ci in range(1, nkA):
            stage1(A, "xk", ci, nc.sync)
            if ci == nkA - 1 and nqA > 1:
                stage1(A, "xq", 1, nc.sync)
            stage1(A, "xv", ci, nc.sync)
        if nkA == 1 and nqA > 1:
            stage1(A, "xq", 1, nc.sync)
        for ci in range(2, nqA):
            stage1(A, "xq", ci, nc.sync)
        if Bp is not None:
            gate = P["w"].tile([1, 1], BF16, tag="bgate", name="bgate")
            gcol = min(512, A["Kp"] - 1)
            nc.gpsimd.tensor_copy(gate[:], A["kc"][0:1, gcol:gcol + 1])
            for ci in range(len(Bp["kch"])):
                stage1(Bp, "xk", ci, nc.gpsimd)
                stage1(Bp, "xv", ci, nc.gpsimd)
            for ci in range(len(Bp["qch"])):
                stage1(Bp, "xq", ci, nc.gpsimd)

        em = _Emitter(nc, P, wts)

        # -------- phase A flow --------
        em.kproj_chunk(A, 0)
        em.qproj_chunk(A, 0)
        rest = deque(_phase_units(em, Bp)) if Bp is not None else deque()
        for ci in range(nqA):
            if ci == 0:
                due = _chunk0_due(em, A)
                anytime = deque()
            else:
                due = deque()
                if ci + 1 < nqA:
                    due.append((max(0, A["NK"] - 3),
                                lambda ci=ci: em.qproj_chunk(A, ci + 1)))
                # B's fillers from chunk 2 on (their DMA lands after A's)
                anytime = rest if ci >= 2 else deque()
            otd = em.ladder(A, ci, due, anytime)
            em.epilogue(A, ci, otd)
            c0, n = A["qch"][ci]
            nc.sync.dma_start(A["io"]["out"][:, :, c0:c0 + n],
                              A["OTs"][:, :, c0:c0 + n])

        # -------- phase B flow --------
        if Bp is not None:
            while rest:
                rest.popleft()()
            if 0 not in Bp["qcs"]:
                em.qproj_chunk(Bp, 0)
            for ci in range(len(Bp["qch"])):
                otd = em.ladder(Bp, ci, deque(), deque())
                if ci + 1 < len(Bp["qch"]):
                    em.qproj_chunk(Bp, ci + 1)
                em.epilogue(Bp, ci, otd)
                c0, n = Bp["qch"][ci]
                nc.sync.dma_start(Bp["io"]["out"][:, :, c0:c0 + n],
                                  Bp["OTs"][:, :, c0:c0 + n])

    nc.compile()
    return nc


def _prep_xT(X, Pq):
    """[T, D] -> [128, KCH, Pq] bf16 with x[p, k, t] = X[t, k*128 + p]."""
    Xp = np.ascontiguousarray(X[:Pq].T)                 # [D, Pq]
    return np.ascontiguousarray(
        Xp.reshape(KCH, 128, Pq).transpose(1, 0, 2)).astype(BNP)


def _prep_w(W, c):
    """[D, H*DH] -> per-core [128, KCH, 128] bf16 slice of heads (2c, 2c+1)."""
    Ws = W[:, c * 128:(c + 1) * 128]                    # [D, 128]
    return np.ascontiguousarray(
        Ws.reshape(KCH, 128, 128).transpose(1, 0, 2)).astype(BNP)


def kernel(Q_seq, K_seq, V_seq, Q_len, V_len, WQ, WK, WV):
    global LAST_EXEC_NS
    Q_seq = np.asarray(Q_seq, dtype=np.float32)
    K_seq = np.asarray(K_seq, dtype=np.float32)
    V_seq = np.asarray(V_seq, dtype=np.float32)
    WQ = np.asarray(WQ, dtype=np.float32)
    WK = np.asarray(WK, dtype=np.float32)
    WV = np.asarray(WV, dtype=np.float32)
    qlen = [int(np.asarray(Q_len)[b, 0]) for b in range(B)]
    vlen = [int(np.asarray(V_len)[b, 0]) for b in range(B)]

    phases = []
    for b in range(B):
        Qp = _ceil_div(qlen[b], 32) * 32   # q only needs 32-elem alignment
        if Qp == 0:
            continue  # whole batch output is zero
        if vlen[b] > 0:
            NK, scale = _ceil_div(vlen[b], 128), SCALE
        else:
            # all keys masked -> reference softmax degenerates to uniform
            # over all T keys; exp(0*S + 0) = 1 reproduces it exactly.
            NK, scale = T // 128, 0.0
        phases.append(dict(b=b, NK=NK, Qp=Qp, Kp=NK * 128, scale=scale))
    phases.sort(key=lambda ph: -ph["Qp"])  # big phase first (filler donor)

    out = np.zeros((B, T, H * DH), dtype=np.float32)
    if not phases:
        return out

    nc = _build_program(phases)

    # per-phase data shared by all cores
    shared = {}
    for ph in phases:
        b, s, Qp, Kp, NK = ph["b"], str(ph["b"]), ph["Qp"], ph["Kp"], ph["NK"]
        kbias = np.where(np.arange(Kp) < vlen[b], 0.0,
                         -NEG_BIG if vlen[b] > 0 else 0.0)
        kbias = np.ascontiguousarray(
            kbias.astype(np.float32).reshape(NK, 128).T)        # [128, NK]
        d = {"kb" + s: kbias}
        for key, X, Pq in (("xq", Q_seq[b], Qp), ("xk", K_seq[b], Kp),
                           ("xv", V_seq[b], Kp)):
            full = _prep_xT(X, Pq)                              # [128, KCH, Pq]
            for ci, (c0, n) in enumerate(_chunks(Pq)):
                d[f"{key}{s}c{ci}"] = np.ascontiguousarray(
                    full[:, :, c0:c0 + n])
        shared[s] = d

    in_maps = []
    for c in range(N_CORES):
        m = {}
        for ph in phases:
            m.update(shared[str(ph["b"])])
        m["wq"] = _prep_w(WQ, c)
        m["wk"] = _prep_w(WK, c)
        m["wv"] = _prep_w(WV, c)
        in_maps.append(m)

    trace = bool(os.environ.get("BASS_TRACE"))
    if trace:
        _ensure_ntff_hook()
    res = run_bass_kernel_spmd(nc, in_maps, list(range(N_CORES)), trace=trace)
    LAST_EXEC_NS = res.exec_time_ns

    for c in range(N_CORES):
        r = res.results[c]
        for ph in phases:
            b, s, ql = ph["b"], str(ph["b"]), qlen[ph["b"]]
            o = np.asarray(r["out" + s]).astype(np.float32)  # [64, 2, Qp]
            for h in (0, 1):
                head = 2 * c + h
                out[b, :ql, head * DH:(head + 1) * DH] = o[:, h, :ql].T
    return out


# revision 32
# speedup vs baseline: 1.2568x; 1.0072x over previous
"""Trainium2 Bass kernel: masked multi-head attention, sharded across 8 NeuronCores.

Problem shapes (hardcoded): B=2, T=2048, D=1024, H=16 heads, dh=64.

Sharding: one SPMD program with two phases (one per batch element). In each
phase every core handles 2 of the 16 heads (core c -> heads 2c, 2c+1), so the
16 heads of each batch are spread over all 8 cores. This load-balances the
data-dependent work (Q_len/V_len trim the q/k tile counts per batch).

v2 changes vs the fp32 baseline:
  - bf16 inputs/weights/intermediates: matmuls run at 1 cycle/row instead of
    fp32's 4 (fp32 lowers to 2 half-speed passes on TRN2), DMA bytes halve.
  - The two heads' S^T matmuls (K=64 each) are row-tiled to disjoint PE
    quadrants (tile_position (0,0)/(64,0)) so they execute concurrently.
  - exp() for both heads merged into one ACT instruction over a 2-bank PSUM
    tile [128, 2, n] (ACT is the #2 engine; fewer/larger instrs).
  - Epilogue: numerator copied once (DVE), softmax denominator row pulled out
    of PSUM by a tiny DMA, reciprocal_approx_fast on DVE (the old
    single-lane RECIPROCAL was 2.2us/chunk), broadcast over partitions with a
    K=1 f32r matmul, one fused multiply per head.
  - Query-length masking moved to the host gather (rows >= Q_len are simply
    not copied out; the output buffer is pre-zeroed) - no qmask work on HW.
  - The second batch's projections are emitted as filler units inside the
    first batch's ACT-paced attention ladder to keep the PE busy.
"""

import math
import os
from collections import deque
from contextlib import ExitStack

import numpy as np
import ml_dtypes

import concourse.bacc as bacc
import concourse.mybir as mybir
import concourse.tile as tile
from concourse.bass_utils import run_bass_kernel_spmd

F32 = mybir.dt.float32
F32R = mybir.dt.float32r
BF16 = mybir.dt.bfloat16
EXP = mybir.ActivationFunctionType.Exp
BNP = ml_dtypes.bfloat16

B, T, D, H, DH = 2, 2048, 1024, 16, 64
N_CORES = 8
KCH = D // 128          # 8 contraction chunks of the model dim
NEG_BIG = 1.0e12
SCALE = 1.0 / math.sqrt(DH)

LAST_EXEC_NS = None     # filled when BASS_TRACE=1


def _ensure_ntff_hook():
    """run_bass_kernel_spmd(trace=True) imports antenv.axon_hooks, which some
    containers lack; synthesize it (backed by libaxon_pjrt's NRT profiling)
    so tracing degrades gracefully instead of crashing."""
    import sys
    import types
    try:
        import antenv.axon_hooks  # noqa: F401
        return
    except ImportError:
        pass
    try:
        import antenv
        from trn_agent_boot.trn_boot import _ntff_profile_via_ctypes
        hook = _ntff_profile_via_ctypes("/opt/axon/libaxon_pjrt.so")
    except Exception:
        antenv = None
        hook = None
    try:
        m = types.ModuleType("antenv.axon_hooks")
        m._hook = hook
        m.set_axon_ntff_profile_hook = lambda h: setattr(m, "_hook", h)
        m.get_axon_ntff_profile_hook = lambda: m._hook
        sys.modules["antenv.axon_hooks"] = m
        if antenv is not None:
            antenv.axon_hooks = m
    except Exception:
        pass


def _ceil_div(a, b):
    return -(-a // b)


def _chunks(total, w=512):
    out = []
    c = 0
    while c < total:
        out.append((c, min(w, total - c)))
        c += w
    return out


class _Emitter:
    def __init__(self, nc, P, wts):
        self.nc = nc
        self.P = P
        self.wts = wts

    # ---------- projection units (each returns nothing, emits instrs) ------

    def kproj_chunk(self, ph, ci, kr=(0, KCH), st=None):
        """Project keys chunk ci: kc[:, c0:c0+n] = (WK.T @ K_seq.T) slice.
        `kr` bounds the contraction range so a chunk can be emitted as two
        filler halves sharing the psum tile passed via `st`."""
        nc, P = self.nc, self.P
        c0, n = ph["kch"][ci]
        xt = ph["xk_tiles"][ci]
        if kr[0] == 0:
            ps = P["pp"].tile([128, 512], F32, tag="pp", name="kps")
            if st is not None:
                st["ps"] = ps
        else:
            ps = st["ps"]
        for k in range(*kr):
            nc.tensor.matmul(ps[:, :n], lhsT=self.wts["wk"][:, k, :],
                             rhs=xt[:, k, :n],
                             start=(k == 0), stop=(k == KCH - 1),
                             skip_group_check=True)
        if kr[1] == KCH:
            nc.vector.tensor_copy(ph["kc"][:, c0:c0 + n], ps[:, :n])

    def vproj_tile(self, ph, m):
        """Project value tokens [m*128,(m+1)*128) into va[:, m, :, 0:64]."""
        nc, P = self.nc, self.P
        ci, r = divmod(m * 128, 512)
        c0, cn = ph["vch"][ci]
        xt = ph["xv_tiles"][ci]
        ps = P["pp"].tile([128, 512], F32, tag="pp", name="vps")
        for k in range(KCH):
            nc.tensor.matmul(ps[:, 0:128], lhsT=xt[:, k, r:r + 128],
                             rhs=self.wts["wv"][:, k, :],
                             start=(k == 0), stop=(k == KCH - 1),
                             skip_group_check=True)
        nc.vector.tensor_copy(
            ph["va"][:, m, :, 0:64],
            ps[:, 0:128].rearrange("p (g d) -> p g d", g=2))

    def qproj_chunk(self, ph, ci):
        """Project queries chunk ci into the qc ring; returns the tile."""
        nc, P = self.nc, self.P
        c0, n = ph["qch"][ci]
        xt = ph["xq_tiles"][ci]
        ps = P["pp"].tile([128, 512], F32, tag="pp", name="qps")
        for k in range(KCH):
            nc.tensor.matmul(ps[:, :n], lhsT=self.wts["wq"][:, k, :],
                             rhs=xt[:, k, :n],
                             start=(k == 0), stop=(k == KCH - 1),
                             skip_group_check=True)
        qc = P["qc"].tile([128, 512], BF16, tag="qc" + str(ph["b"]),
                          name="qc", bufs=3)
        nc.vector.tensor_copy(qc[:, :n], ps[:, :n])
        ph["qcs"][ci] = qc
        return qc

    # ---------- attention ladder ------------------------------------------

    def ladder(self, ph, ci, due, anytime):
        """S/exp/PV software pipeline for q chunk ci.

        `due`: deque of (deadline_step, closure) in non-decreasing deadline
        order — every unit whose deadline has arrived is emitted that step
        (these carry dataflow deadlines, e.g. vproj(kt) before PV(kt)).
        `anytime`: deque of independent filler closures; at most one is
        popped per step, only on steps with no due unit (keeps PE work per
        step under the ACT exp cadence)."""
        nc, P = self.nc, self.P
        c0, n = ph["qch"][ci]
        NK = ph["NK"]
        qc = ph["qcs"].pop(ci)
        kb = ph["kb_tile"]
        kc, va = ph["kc"], ph["va"]
        scale = ph["scale"]

        otd = P["ot"].tile([65, 2, 512], F32, tag="ot", name="otd")

        def emit_s(kt):
            sps = P["sp"].tile([128, 2, 512], F32, tag="sp", name="sps")
            for h in (0, 1):
                nc.tensor.matmul(
                    sps[:, h, :n],
                    lhsT=kc[h * 64:(h + 1) * 64, kt * 128:(kt + 1) * 128],
                    rhs=qc[h * 64:(h + 1) * 64, :n],
                    start=True, stop=True,
                    tile_position=(h * 64, 0),
                    skip_group_check=True)
            e = P["e"].tile([128, 2, 512], BF16, tag="e", name="e", bufs=3)
            nc.scalar.activation(e[:, :, :n], sps[:, :, :n], EXP,
                                 bias=kb[:, kt:kt + 1], scale=scale)
            return e

        ep = emit_s(0)
        for kt in range(NK):
            ec = ep
            if kt + 1 < NK:
                ep = emit_s(kt + 1)
            popped = False
            while due and due[0][0] <= kt:
                due.popleft()[1]()
                popped = True
            if not popped and anytime:
                anytime.popleft()()
            for h in (0, 1):
                nc.tensor.matmul(otd[:, h, :n], lhsT=va[:, kt, h, :],
                                 rhs=ec[:, h, :n],
                                 start=(kt == 0), stop=(kt == NK - 1),
                                 skip_group_check=True)
        return otd

    def epilogue(self, ph, ci, otd):
        """Normalize otd -> OTs[:, :, c0:c0+n] (no qmask: host trims)."""
        nc, P = self.nc, self.P
        c0, n = ph["qch"][ci]
        ou = P["ou"].tile([64, 2, 512], BF16, tag="ou", name="ou", bufs=2)
        nc.vector.tensor_copy(ou[:, :, :n], otd[0:64, :, :n])
        drow = P["rows"].tile([65, 2, 512], BF16, tag="drow", name="drow",
                              bufs=2)
        nc.vector.tensor_copy(drow[64:65, :, :n], otd[64:65, :, :n])
        rsb = P["rows"].tile([64, 2, 512], F32, tag="rsb", name="rsb",
                             bufs=2)
        for h in (0, 1):
            # broadcast d over 64 partitions (K=1 bf16 matmul), then
            # reciprocal on the [64, n] block (DVE cost is free-size-based,
            # so this is no dearer than a single-partition reciprocal).
            dps = P["pp"].tile([128, 512], F32, tag="pp", name="dps")
            nc.tensor.matmul(dps[0:64, :n],
                             lhsT=P["onesr"][64:65, 0:64],
                             rhs=drow[64:65, h, :n],
                             start=True, stop=True, skip_group_check=True)
            nc.vector.reciprocal_approx_fast(rsb[:, h, :n], dps[0:64, :n])
            nc.vector.tensor_mul(ph["OTs"][:, h, c0:c0 + n],
                                 ou[:, h, :n], rsb[:, h, :n])


def _chunk0_due(em, ph):
    """Deadline units for the first q chunk's ladder: the phase's remaining
    k-proj chunks and all v-proj tiles, interleaved in the exact order their
    DMA chunks arrive, plus qproj(1). Deadlines: vproj(m) before PV(m)
    (popped a step early so the DVE copy hides), kproj(ci) before S(4ci)
    which is emitted at step 4ci-1, qproj(1) a few steps before chunk end."""
    NK = ph["NK"]
    due = []
    for m in range(NK):
        if m >= 1 and m % 4 == 0:
            due.append((m - 2, lambda ci=m // 4: em.kproj_chunk(ph, ci)))
        due.append((max(0, m - 1), lambda m=m: em.vproj_tile(ph, m)))
    if len(ph["qch"]) > 1:
        d = max(0, NK - 3)
        pos = next((i for i, u in enumerate(due) if u[0] > d), len(due))
        due.insert(pos, (d, lambda: em.qproj_chunk(ph, 1)))
    return deque(due)


def _phase_units(em, ph):
    """Independent filler closures projecting all of phase `ph`'s inputs,
    in DMA-arrival order. kproj chunks are split in two halves so a single
    pop stays under the ladder's per-step PE budget."""
    units = []
    for ci in range(len(ph["kch"])):
        st = {}
        units.append(lambda ci=ci, st=st:
                     em.kproj_chunk(ph, ci, kr=(0, KCH // 2), st=st))
        units.append(lambda ci=ci, st=st:
                     em.kproj_chunk(ph, ci, kr=(KCH // 2, KCH), st=st))
        for m in range(ci * 4, min((ci + 1) * 4, ph["NK"])):
            units.append(lambda m=m: em.vproj_tile(ph, m))
    units.append(lambda: em.qproj_chunk(ph, 0))
    return units


def _build_program(phases):
    nc = bacc.Bacc("TRN2", target_bir_lowering=False, debug=False,
                   num_devices=N_CORES)
    for ph in phases:
        s = str(ph["b"])
        Qp, Kp, NK = ph["Qp"], ph["Kp"], ph["NK"]
        ph["qch"] = _chunks(Qp)
        ph["kch"] = _chunks(Kp)
        ph["vch"] = ph["kch"]
        ph["qcs"] = {}
        io = {
            "kb": nc.dram_tensor("kb" + s, [128, NK], F32, kind="ExternalInput"),
            "out": nc.dram_tensor("out" + s, [64, 2, Qp], BF16, kind="ExternalOutput"),
        }
        # per-chunk input tensors: per-partition-contiguous so each DMA
        # lowers to 128 large descriptors instead of 1KB-strided fragments
        for key, chl in (("xq", ph["qch"]), ("xk", ph["kch"]), ("xv", ph["vch"])):
            for ci, (c0, n) in enumerate(chl):
                io[f"{key}c{ci}"] = nc.dram_tensor(
                    f"{key}{s}c{ci}", [128, KCH, n], BF16, kind="ExternalInput")
        ph["io"] = io

    with tile.TileContext(nc) as tc, ExitStack() as ctx:
        P = {
            "w": ctx.enter_context(tc.tile_pool(name="w", bufs=1)),
            "x": ctx.enter_context(tc.tile_pool(name="x", bufs=1)),
            "qc": ctx.enter_context(tc.tile_pool(name="qc", bufs=3)),
            "e": ctx.enter_context(tc.tile_pool(name="e", bufs=3)),
            "ou": ctx.enter_context(tc.tile_pool(name="ou", bufs=2)),
            "rows": ctx.enter_context(tc.tile_pool(name="rows", bufs=2)),
            "persist": ctx.enter_context(tc.tile_pool(name="persist", bufs=1)),
            "pp": ctx.enter_context(tc.tile_pool(name="pp", bufs=2, space="PSUM")),
            "sp": ctx.enter_context(tc.tile_pool(name="sp", bufs=2, space="PSUM")),
            "ot": ctx.enter_context(tc.tile_pool(name="ot", bufs=1, space="PSUM")),
        }
        onesr = P["w"].tile([65, 64], BF16, tag="onesr", name="onesr")
        nc.vector.memset(onesr[64:65, :], 1.0)
        P["onesr"] = onesr
        warm = P["w"].tile([1, 1], F32, tag="actwarm", name="actwarm")
        nc.vector.memset(warm[:], 0.0)
        nc.scalar.activation(warm[:], warm[:], EXP)

        # PE p-state warmup: dummy bf16 matmuls on zeroed tiles keep the PE
        # clocking up while the first input DMAs land.
        zw = P["w"].tile([128, 128], BF16, tag="zw", name="zw")
        nc.gpsimd.memset(zw[:], 0.0)
        zw2 = P["w"].tile([128, 512], BF16, tag="zw2", name="zw2")
        nc.gpsimd.memset(zw2[:], 0.0)
        for _ in range(2):
            wps = P["sp"].tile([128, 2, 512], F32, tag="sp", name="wps")
            for r in range(4):
                nc.tensor.matmul(wps[:, 0, :], lhsT=zw[:], rhs=zw2[:],
                                 start=(r == 0), stop=(r == 3),
                                 skip_group_check=True)

        # -------- weights --------
        wts = {}
        for nm in ("wk", "wq", "wv"):
            wts[nm] = nc.dram_tensor(nm, [128, KCH, 128], BF16,
                                     kind="ExternalInput")

        def load_w(nm):
            t = P["w"].tile([128, KCH, 128], BF16, tag=nm, name=nm)
            nc.sync.dma_start(t[:], wts[nm][:])
            wts[nm] = t

        # -------- input staging (issue order == consumption order) --------
        A = phases[0]
        Bp = phases[1] if len(phases) > 1 else None
        for ph in phases:
            for key, chl in (("xq", ph["qch"]), ("xk", ph["kch"]),
                             ("xv", ph["vch"])):
                ph[f"{key}_tiles"] = [None] * len(chl)

        def load_kb(ph):
            s = str(ph["b"])
            kb = P["w"].tile([128, ph["NK"]], F32, tag="kb" + s, name="kb")
            nc.sync.dma_start(kb[:], ph["io"]["kb"][:])
            ph["kb_tile"] = kb

        def stage1(ph, key, ci, eng, halves=1):
            """One input chunk -> SBUF, issued from `eng` (DMA trigger issue
            is ~0.6us+size serial per issuing sequencer, so spread engines)."""
            s = str(ph["b"])
            n = dict(xq=ph["qch"], xk=ph["kch"], xv=ph["vch"])[key][ci][1]
            xt = P["x"].tile([128, KCH, n], BF16, tag=f"{key}{s}c{ci}",
                             name=f"{key}{s}c{ci}", bufs=1)
            src = ph["io"][f"{key}c{ci}"]
            step = KCH // halves
            for k in range(0, KCH, step):
                eng.dma_start(xt[:, k:k + step, :], src[:, k:k + step, :])
            ph[f"{key}_tiles"][ci] = xt

        # -------- persistent per-phase tiles --------
        for ph in phases:
            s = str(ph["b"])
            ph["kc"] = P["persist"].tile([128, ph["Kp"]], BF16,
                                         tag="kc" + s, name="kc" + s)
            ph["va"] = P["persist"].tile([128, ph["NK"], 2, 65], BF16,
                                         tag="va" + s, name="va" + s)
            nc.gpsimd.memset(ph["va"][:, :, :, 64:65], 1.0)
            ph["OTs"] = P["persist"].tile([64, 2, ph["Qp"]], BF16,
                                          tag="oT" + s, name="oT" + s)

        # All of phase A's DMA is issued serially from SP in exact
        # consumption order: issue order is the only priority mechanism the
        # 16 shared queues honor, and ring backpressure then throttles SP
        # naturally. Phase B is staged later (inside the chunk-1 emission)
        # on GpSimd software-DGE behind a dependency gate.
        nkA, nqA = len(A["kch"]), len(A["qch"])
        load_w("wk")
        stage1(A, "xk", 0, nc.sync, halves=2)
        load_w("wq")
        stage1(A, "xq", 0, nc.sync)
        load_kb(A)
        if Bp is not None:
            load_kb(Bp)
        load_w("wv")
        stage1(A, "xv", 0, nc.sync)
        for ci in range(1, nkA):
            stage1(A, "xk", ci, nc.sync)
            if ci == nkA - 1 and nqA > 1:
                stage1(A, "xq", 1, nc.sync)
            stage1(A, "xv", ci, nc.sync)
        if nkA == 1 and nqA > 1:
            stage1(A, "xq", 1, nc.sync)
        for ci in range(2, nqA):
            stage1(A, "xq", ci, nc.sync)

        def stage_B():
            # emitted after chunk-0's ladder so the gate's dependency on
            # kc (written by the in-ladder kproj fillers) is known to Tile
            gate = P["w"].tile([1, 1], BF16, tag="bgate", name="bgate")
            gcol = min(512, A["Kp"] - 1)
            nc.gpsimd.tensor_copy(gate[:], A["kc"][0:1, gcol:gcol + 1])
            for ci in range(len(Bp["kch"])):
                stage1(Bp, "xk", ci, nc.gpsimd)
                stage1(Bp, "xv", ci, nc.gpsimd)
            for ci in range(len(Bp["qch"])):
                stage1(Bp, "xq", ci, nc.gpsimd)

        em = _Emitter(nc, P, wts)

        # -------- phase A flow --------
        em.kproj_chunk(A, 0)
        em.qproj_chunk(A, 0)
        rest = deque(_phase_units(em, Bp)) if Bp is not None else deque()
        for ci in range(nqA):
            if ci == 1 and Bp is not None:
                stage_B()
            if ci == 0:
                due = _chunk0_due(em, A)
                anytime = deque()
            else:
                due = deque()
                if ci + 1 < nqA:
                    due.append((max(0, A["NK"] - 3),
                                lambda ci=ci: em.qproj_chunk(A, ci + 1)))
                # B's fillers from chunk 2 on (their DMA lands after A's)
                anytime = rest if ci >= 2 else deque()
            otd = em.ladder(A, ci, due, anytime)
            em.epilogue(A, ci, otd)
            c0, n = A["qch"][ci]
            nc.sync.dma_start(A["io"]["out"][:, :, c0:c0 + n],
                              A["OTs"][:, :, c0:c0 + n])

        # -------- phase B flow --------
        if Bp is not None:
            if Bp["xk_tiles"][0] is None:
                stage_B()
            while rest:
                rest.popleft()()
            if 0 not in Bp["qcs"]:
                em.qproj_chunk(Bp, 0)
            for ci in range(len(Bp["qch"])):
                otd = em.ladder(Bp, ci, deque(), deque())
                if ci + 1 < len(Bp["qch"]):
                    em.qproj_chunk(Bp, ci + 1)
                em.epilogue(Bp, ci, otd)
                c0, n = Bp["qch"][ci]
                nc.sync.dma_start(Bp["io"]["out"][:, :, c0:c0 + n],
                                  Bp["OTs"][:, :, c0:c0 + n])

    nc.compile()
    return nc


def _prep_xT(X, Pq):
    """[T, D] -> [128, KCH, Pq] bf16 with x[p, k, t] = X[t, k*128 + p]."""
    Xp = np.ascontiguousarray(X[:Pq].T)                 # [D, Pq]
    return np.ascontiguousarray(
        Xp.reshape(KCH, 128, Pq).transpose(1, 0, 2)).astype(BNP)


def _prep_w(W, c):
    """[D, H*DH] -> per-core [128, KCH, 128] bf16 slice of heads (2c, 2c+1)."""
    Ws = W[:, c * 128:(c + 1) * 128]                    # [D, 128]
    return np.ascontiguousarray(
        Ws.reshape(KCH, 128, 128).transpose(1, 0, 2)).astype(BNP)


def kernel(Q_seq, K_seq, V_seq, Q_len, V_len, WQ, WK, WV):
    global LAST_EXEC_NS
    Q_seq = np.asarray(Q_seq, dtype=np.float32)
    K_seq = np.asarray(K_seq, dtype=np.float32)
    V_seq = np.asarray(V_seq, dtype=np.float32)
    WQ = np.asarray(WQ, dtype=np.float32)
    WK = np.asarray(WK, dtype=np.float32)
    WV = np.asarray(WV, dtype=np.float32)
    qlen = [int(np.asarray(Q_len)[b, 0]) for b in range(B)]
    vlen = [int(np.asarray(V_len)[b, 0]) for b in range(B)]

    phases = []
    for b in range(B):
        Qp = _ceil_div(qlen[b], 32) * 32   # q only needs 32-elem alignment
        if Qp == 0:
            continue  # whole batch output is zero
        if vlen[b] > 0:
            NK, scale = _ceil_div(vlen[b], 128), SCALE
        else:
            # all keys masked -> reference softmax degenerates to uniform
            # over all T keys; exp(0*S + 0) = 1 reproduces it exactly.
            NK, scale = T // 128, 0.0
        phases.append(dict(b=b, NK=NK, Qp=Qp, Kp=NK * 128, scale=scale))
    phases.sort(key=lambda ph: -ph["Qp"])  # big phase first (filler donor)

    out = np.zeros((B, T, H * DH), dtype=np.float32)
    if not phases:
        return out

    nc = _build_program(phases)

    # per-phase data shared by all cores
    shared = {}
    for ph in phases:
        b, s, Qp, Kp, NK = ph["b"], str(ph["b"]), ph["Qp"], ph["Kp"], ph["NK"]
        kbias = np.where(np.arange(Kp) < vlen[b], 0.0,
                         -NEG_BIG if vlen[b] > 0 else 0.0)
        kbias = np.ascontiguousarray(
            kbias.astype(np.float32).reshape(NK, 128).T)        # [128, NK]
        d = {"kb" + s: kbias}
        for key, X, Pq in (("xq", Q_seq[b], Qp), ("xk", K_seq[b], Kp),
                           ("xv", V_seq[b], Kp)):
            full = _prep_xT(X, Pq)                              # [128, KCH, Pq]
            for ci, (c0, n) in enumerate(_chunks(Pq)):
                d[f"{key}{s}c{ci}"] = np.ascontiguousarray(
                    full[:, :, c0:c0 + n])
        shared[s] = d

    in_maps = []
    for c in range(N_CORES):
        m = {}
        for ph in phases:
            m.update(shared[str(ph["b"])])
        m["wq"] = _prep_w(WQ, c)
        m["wk"] = _prep_w(WK, c)
        m["wv"] = _prep_w(WV, c)
        in_maps.append(m)

    trace = bool(os.environ.get("BASS_TRACE"))
    if trace:
        _ensure_ntff_hook()
    res = run_bass_kernel_spmd(nc, in_maps, list(range(N_CORES)), trace=trace)
    LAST_EXEC_NS = res.exec_time_ns

    for c in range(N_CORES):
        r = res.results[c]
        for ph in phases:
            b, s, ql = ph["b"], str(ph["b"]), qlen[ph["b"]]
            o = np.asarray(r["out" + s]).astype(np.float32)  # [64, 2, Qp]
            for h in (0, 1):
                head = 2 * c + h
                out[b, :ql, head * DH:(head + 1) * DH] = o[:, h, :ql].T
    return out


# revision 33
# speedup vs baseline: 1.3650x; 1.0861x over previous
"""Trainium2 Bass kernel: masked multi-head attention, sharded across 8 NeuronCores.

Problem shapes (hardcoded): B=2, T=2048, D=1024, H=16 heads, dh=64.

Sharding: one SPMD program with two phases (one per batch element). In each
phase every core handles 2 of the 16 heads (core c -> heads 2c, 2c+1), so the
16 heads of each batch are spread over all 8 cores. This load-balances the
data-dependent work (Q_len/V_len trim the q/k tile counts per batch).

v2 changes vs the fp32 baseline:
  - bf16 inputs/weights/intermediates: matmuls run at 1 cycle/row instead of
    fp32's 4 (fp32 lowers to 2 half-speed passes on TRN2), DMA bytes halve.
  - The two heads' S^T matmuls (K=64 each) are row-tiled to disjoint PE
    quadrants (tile_position (0,0)/(64,0)) so they execute concurrently.
  - exp() for both heads merged into one ACT instruction over a 2-bank PSUM
    tile [128, 2, n] (ACT is the #2 engine; fewer/larger instrs).
  - Epilogue: numerator copied once (DVE), softmax denominator row pulled out
    of PSUM by a tiny DMA, reciprocal_approx_fast on DVE (the old
    single-lane RECIPROCAL was 2.2us/chunk), broadcast over partitions with a
    K=1 f32r matmul, one fused multiply per head.
  - Query-length masking moved to the host gather (rows >= Q_len are simply
    not copied out; the output buffer is pre-zeroed) - no qmask work on HW.
  - The second batch's projections are emitted as filler units inside the
    first batch's ACT-paced attention ladder to keep the PE busy.
"""

import math
import os
from collections import deque
from contextlib import ExitStack

import numpy as np
import ml_dtypes

import concourse.bacc as bacc
import concourse.mybir as mybir
import concourse.tile as tile
from concourse.bass_utils import run_bass_kernel_spmd

F32 = mybir.dt.float32
F32R = mybir.dt.float32r
BF16 = mybir.dt.bfloat16
EXP = mybir.ActivationFunctionType.Exp
BNP = ml_dtypes.bfloat16

B, T, D, H, DH = 2, 2048, 1024, 16, 64
N_CORES = 8
KCH = D // 128          # 8 contraction chunks of the model dim
NEG_BIG = 1.0e12
SCALE = 1.0 / math.sqrt(DH)

LAST_EXEC_NS = None     # filled when BASS_TRACE=1


def _ensure_ntff_hook():
    """run_bass_kernel_spmd(trace=True) imports antenv.axon_hooks, which some
    containers lack; synthesize it (backed by libaxon_pjrt's NRT profiling)
    so tracing degrades gracefully instead of crashing."""
    import sys
    import types
    try:
        import antenv.axon_hooks  # noqa: F401
        return
    except ImportError:
        pass
    try:
        import antenv
        from trn_agent_boot.trn_boot import _ntff_profile_via_ctypes
        hook = _ntff_profile_via_ctypes("/opt/axon/libaxon_pjrt.so")
    except Exception:
        antenv = None
        hook = None
    try:
        m = types.ModuleType("antenv.axon_hooks")
        m._hook = hook
        m.set_axon_ntff_profile_hook = lambda h: setattr(m, "_hook", h)
        m.get_axon_ntff_profile_hook = lambda: m._hook
        sys.modules["antenv.axon_hooks"] = m
        if antenv is not None:
            antenv.axon_hooks = m
    except Exception:
        pass


def _ceil_div(a, b):
    return -(-a // b)


def _chunks(total, w=512):
    out = []
    c = 0
    while c < total:
        out.append((c, min(w, total - c)))
        c += w
    return out


class _Emitter:
    def __init__(self, nc, P, wts):
        self.nc = nc
        self.P = P
        self.wts = wts

    # ---------- projection units (each returns nothing, emits instrs) ------

    def kproj_chunk(self, ph, ci, kr=(0, KCH), st=None):
        """Project keys chunk ci: kc[:, c0:c0+n] = (WK.T @ K_seq.T) slice.
        `kr` bounds the contraction range so a chunk can be emitted as two
        filler halves sharing the psum tile passed via `st`."""
        nc, P = self.nc, self.P
        c0, n = ph["kch"][ci]
        xt = ph["xk_tiles"][ci]
        if kr[0] == 0:
            ps = P["pp"].tile([128, 512], F32, tag="pp", name="kps")
            if st is not None:
                st["ps"] = ps
        else:
            ps = st["ps"]
        for k in range(*kr):
            nc.tensor.matmul(ps[:, :n], lhsT=self.wts["wk"][:, k, :],
                             rhs=xt[:, k, :n],
                             start=(k == 0), stop=(k == KCH - 1),
                             skip_group_check=True)
        if kr[1] == KCH:
            nc.vector.tensor_copy(ph["kc"][:, c0:c0 + n], ps[:, :n])

    def vproj_tile(self, ph, m):
        """Project value tokens [m*128,(m+1)*128) into va[:, m, :, 0:64]."""
        nc, P = self.nc, self.P
        ci, r = divmod(m * 128, 512)
        c0, cn = ph["vch"][ci]
        xt = ph["xv_tiles"][ci]
        ps = P["pp"].tile([128, 512], F32, tag="pp", name="vps")
        for k in range(KCH):
            nc.tensor.matmul(ps[:, 0:128], lhsT=xt[:, k, r:r + 128],
                             rhs=self.wts["wv"][:, k, :],
                             start=(k == 0), stop=(k == KCH - 1),
                             skip_group_check=True)
        nc.vector.tensor_copy(
            ph["va"][:, m, :, 0:64],
            ps[:, 0:128].rearrange("p (g d) -> p g d", g=2))

    def qproj_chunk(self, ph, ci):
        """Project queries chunk ci into the qc ring; returns the tile."""
        nc, P = self.nc, self.P
        c0, n = ph["qch"][ci]
        xt = ph["xq_tiles"][ci]
        ps = P["pp"].tile([128, 512], F32, tag="pp", name="qps")
        for k in range(KCH):
            nc.tensor.matmul(ps[:, :n], lhsT=self.wts["wq"][:, k, :],
                             rhs=xt[:, k, :n],
                             start=(k == 0), stop=(k == KCH - 1),
                             skip_group_check=True)
        qc = P["qc"].tile([128, 512], BF16, tag="qc" + str(ph["b"]),
                          name="qc", bufs=3)
        nc.vector.tensor_copy(qc[:, :n], ps[:, :n])
        ph["qcs"][ci] = qc
        return qc

    # ---------- attention ladder ------------------------------------------

    def ladder(self, ph, ci, due, anytime):
        """S/exp/PV software pipeline for q chunk ci.

        `due`: deque of (deadline_step, closure) in non-decreasing deadline
        order — every unit whose deadline has arrived is emitted that step
        (these carry dataflow deadlines, e.g. vproj(kt) before PV(kt)).
        `anytime`: deque of independent filler closures; at most one is
        popped per step, only on steps with no due unit (keeps PE work per
        step under the ACT exp cadence)."""
        nc, P = self.nc, self.P
        c0, n = ph["qch"][ci]
        NK = ph["NK"]
        qc = ph["qcs"].pop(ci)
        kb = ph["kb_tile"]
        kc, va = ph["kc"], ph["va"]
        scale = ph["scale"]

        otd = P["ot"].tile([65, 2, 512], F32, tag="ot", name="otd")

        def emit_s(kt):
            sps = P["sp"].tile([128, 2, 512], F32, tag="sp", name="sps")
            for h in (0, 1):
                nc.tensor.matmul(
                    sps[:, h, :n],
                    lhsT=kc[h * 64:(h + 1) * 64, kt * 128:(kt + 1) * 128],
                    rhs=qc[h * 64:(h + 1) * 64, :n],
                    start=True, stop=True,
                    tile_position=(h * 64, 0),
                    skip_group_check=True)
            e = P["e"].tile([128, 2, 512], BF16, tag="e", name="e", bufs=3)
            nc.scalar.activation(e[:, :, :n], sps[:, :, :n], EXP,
                                 bias=kb[:, kt:kt + 1], scale=scale)
            return e

        ep = emit_s(0)
        for kt in range(NK):
            ec = ep
            if kt + 1 < NK:
                ep = emit_s(kt + 1)
            popped = False
            while due and due[0][0] <= kt:
                due.popleft()[1]()
                popped = True
            if not popped and anytime:
                anytime.popleft()()
            for h in (0, 1):
                nc.tensor.matmul(otd[:, h, :n], lhsT=va[:, kt, h, :],
                                 rhs=ec[:, h, :n],
                                 start=(kt == 0), stop=(kt == NK - 1),
                                 skip_group_check=True)
        return otd

    def epilogue(self, ph, ci, otd):
        """Normalize otd -> OTs[:, :, c0:c0+n] (no qmask: host trims)."""
        nc, P = self.nc, self.P
        c0, n = ph["qch"][ci]
        ou = P["ou"].tile([64, 2, 512], BF16, tag="ou", name="ou", bufs=2)
        nc.vector.tensor_copy(ou[:, :, :n], otd[0:64, :, :n])
        drow = P["rows"].tile([65, 2, 512], BF16, tag="drow", name="drow",
                              bufs=2)
        nc.vector.tensor_copy(drow[64:65, :, :n], otd[64:65, :, :n])
        rsb = P["rows"].tile([64, 2, 512], F32, tag="rsb", name="rsb",
                             bufs=2)
        for h in (0, 1):
            # broadcast d over 64 partitions (K=1 bf16 matmul), then
            # reciprocal on the [64, n] block (DVE cost is free-size-based,
            # so this is no dearer than a single-partition reciprocal).
            dps = P["pp"].tile([128, 512], F32, tag="pp", name="dps")
            nc.tensor.matmul(dps[0:64, :n],
                             lhsT=P["onesr"][64:65, 0:64],
                             rhs=drow[64:65, h, :n],
                             start=True, stop=True, skip_group_check=True)
            nc.vector.reciprocal_approx_fast(rsb[:, h, :n], dps[0:64, :n])
            nc.vector.tensor_mul(ph["OTs"][:, h, c0:c0 + n],
                                 ou[:, h, :n], rsb[:, h, :n])


def _chunk0_due(em, ph):
    """Deadline units for the first q chunk's ladder: the phase's remaining
    k-proj chunks and all v-proj tiles, interleaved in the exact order their
    DMA chunks arrive, plus qproj(1). Deadlines: vproj(m) before PV(m)
    (popped a step early so the DVE copy hides), kproj(ci) before S(4ci)
    which is emitted at step 4ci-1, qproj(1) a few steps before chunk end."""
    NK = ph["NK"]
    due = []
    for m in range(NK):
        if m >= 1 and m % 4 == 0:
            due.append((m - 2, lambda ci=m // 4: em.kproj_chunk(ph, ci)))
        due.append((max(0, m - 1), lambda m=m: em.vproj_tile(ph, m)))
    if len(ph["qch"]) > 1:
        d = max(0, NK - 3)
        pos = next((i for i, u in enumerate(due) if u[0] > d), len(due))
        due.insert(pos, (d, lambda: em.qproj_chunk(ph, 1)))
    return deque(due)


def _phase_units(em, ph):
    """Independent filler closures projecting all of phase `ph`'s inputs,
    in DMA-arrival order. kproj chunks are split in two halves so a single
    pop stays under the ladder's per-step PE budget."""
    units = []
    for ci in range(len(ph["kch"])):
        st = {}
        units.append(lambda ci=ci, st=st:
                     em.kproj_chunk(ph, ci, kr=(0, KCH // 2), st=st))
        units.append(lambda ci=ci, st=st:
                     em.kproj_chunk(ph, ci, kr=(KCH // 2, KCH), st=st))
        for m in range(ci * 4, min((ci + 1) * 4, ph["NK"])):
            units.append(lambda m=m: em.vproj_tile(ph, m))
    units.append(lambda: em.qproj_chunk(ph, 0))
    return units


def _build_program(phases):
    nc = bacc.Bacc("TRN2", target_bir_lowering=False, debug=False,
                   num_devices=N_CORES)
    for ph in phases:
        s = str(ph["b"])
        Qp, Kp, NK = ph["Qp"], ph["Kp"], ph["NK"]
        ph["qch"] = _chunks(Qp)
        ph["kch"] = _chunks(Kp)
        ph["vch"] = ph["kch"]
        ph["qcs"] = {}
        io = {
            "kb": nc.dram_tensor("kb" + s, [128, NK], F32, kind="ExternalInput"),
            "out": nc.dram_tensor("out" + s, [64, 2, Qp], BF16, kind="ExternalOutput"),
        }
        # per-chunk input tensors: per-partition-contiguous so each DMA
        # lowers to 128 large descriptors instead of 1KB-strided fragments
        for key, chl in (("xq", ph["qch"]), ("xk", ph["kch"]), ("xv", ph["vch"])):
            for ci, (c0, n) in enumerate(chl):
                io[f"{key}c{ci}"] = nc.dram_tensor(
                    f"{key}{s}c{ci}", [128, KCH, n], BF16, kind="ExternalInput")
        ph["io"] = io

    with tile.TileContext(nc) as tc, ExitStack() as ctx:
        P = {
            "w": ctx.enter_context(tc.tile_pool(name="w", bufs=1)),
            "x": ctx.enter_context(tc.tile_pool(name="x", bufs=1)),
            "qc": ctx.enter_context(tc.tile_pool(name="qc", bufs=3)),
            "e": ctx.enter_context(tc.tile_pool(name="e", bufs=3)),
            "ou": ctx.enter_context(tc.tile_pool(name="ou", bufs=2)),
            "rows": ctx.enter_context(tc.tile_pool(name="rows", bufs=2)),
            "persist": ctx.enter_context(tc.tile_pool(name="persist", bufs=1)),
            "pp": ctx.enter_context(tc.tile_pool(name="pp", bufs=2, space="PSUM")),
            "sp": ctx.enter_context(tc.tile_pool(name="sp", bufs=2, space="PSUM")),
            "ot": ctx.enter_context(tc.tile_pool(name="ot", bufs=1, space="PSUM")),
        }
        onesr = P["w"].tile([65, 64], BF16, tag="onesr", name="onesr")
        nc.vector.memset(onesr[64:65, :], 1.0)
        P["onesr"] = onesr
        warm = P["w"].tile([1, 1], F32, tag="actwarm", name="actwarm")
        nc.vector.memset(warm[:], 0.0)
        nc.scalar.activation(warm[:], warm[:], EXP)

        # PE p-state warmup: dummy bf16 matmuls on zeroed tiles keep the PE
        # clocking up while the first input DMAs land.
        zw = P["w"].tile([128, 128], BF16, tag="zw", name="zw")
        nc.gpsimd.memset(zw[:], 0.0)
        zw2 = P["w"].tile([128, 512], BF16, tag="zw2", name="zw2")
        nc.gpsimd.memset(zw2[:], 0.0)
        for _ in range(2):
            wps = P["sp"].tile([128, 2, 512], F32, tag="sp", name="wps")
            for r in range(4):
                nc.tensor.matmul(wps[:, 0, :], lhsT=zw[:], rhs=zw2[:],
                                 start=(r == 0), stop=(r == 3),
                                 skip_group_check=True)

        # -------- weights --------
        wts = {}
        for nm in ("wk", "wq", "wv"):
            wts[nm] = nc.dram_tensor(nm, [128, KCH, 128], BF16,
                                     kind="ExternalInput")

        def load_w(nm):
            t = P["w"].tile([128, KCH, 128], BF16, tag=nm, name=nm)
            nc.sync.dma_start(t[:], wts[nm][:])
            wts[nm] = t

        # -------- input staging (issue order == consumption order) --------
        A = phases[0]
        Bp = phases[1] if len(phases) > 1 else None
        for ph in phases:
            for key, chl in (("xq", ph["qch"]), ("xk", ph["kch"]),
                             ("xv", ph["vch"])):
                ph[f"{key}_tiles"] = [None] * len(chl)

        def load_kb(ph):
            s = str(ph["b"])
            kb = P["w"].tile([128, ph["NK"]], F32, tag="kb" + s, name="kb")
            nc.sync.dma_start(kb[:], ph["io"]["kb"][:])
            ph["kb_tile"] = kb

        def stage1(ph, key, ci, eng, halves=1):
            """One input chunk -> SBUF, issued from `eng` (DMA trigger issue
            is ~0.6us+size serial per issuing sequencer, so spread engines)."""
            s = str(ph["b"])
            n = dict(xq=ph["qch"], xk=ph["kch"], xv=ph["vch"])[key][ci][1]
            xt = P["x"].tile([128, KCH, n], BF16, tag=f"{key}{s}c{ci}",
                             name=f"{key}{s}c{ci}", bufs=1)
            src = ph["io"][f"{key}c{ci}"]
            step = KCH // halves
            for k in range(0, KCH, step):
                eng.dma_start(xt[:, k:k + step, :], src[:, k:k + step, :])
            ph[f"{key}_tiles"][ci] = xt

        # -------- persistent per-phase tiles --------
        for ph in phases:
            s = str(ph["b"])
            ph["kc"] = P["persist"].tile([128, ph["Kp"]], BF16,
                                         tag="kc" + s, name="kc" + s)
            ph["va"] = P["persist"].tile([128, ph["NK"], 2, 65], BF16,
                                         tag="va" + s, name="va" + s)
            nc.gpsimd.memset(ph["va"][:, :, :, 64:65], 1.0)
            ph["OTs"] = P["persist"].tile([64, 2, ph["Qp"]], BF16,
                                          tag="oT" + s, name="oT" + s)

        # All of phase A's DMA is issued serially from SP in exact
        # consumption order: issue order is the only priority mechanism the
        # 16 shared queues honor, and ring backpressure then throttles SP
        # naturally. Phase B is staged later (inside the chunk-1 emission)
        # on GpSimd software-DGE behind a dependency gate.
        nkA, nqA = len(A["kch"]), len(A["qch"])
        load_w("wk")
        stage1(A, "xk", 0, nc.sync, halves=2)
        load_w("wq")
        stage1(A, "xq", 0, nc.sync)
        load_kb(A)
        if Bp is not None:
            load_kb(Bp)
        load_w("wv")
        stage1(A, "xv", 0, nc.sync)
        for ci in range(1, nkA):
            stage1(A, "xk", ci, nc.sync)
            if ci == nkA - 1 and nqA > 1:
                stage1(A, "xq", 1, nc.sync)
            stage1(A, "xv", ci, nc.sync)
        if nkA == 1 and nqA > 1:
            stage1(A, "xq", 1, nc.sync)
        for ci in range(2, nqA):
            stage1(A, "xq", ci, nc.sync)

        def stage_B():
            # SP's serial trigger FIFO is the throttle: these fire only
            # after all of phase A's transfers have been enqueued
            for ci in range(len(Bp["kch"])):
                stage1(Bp, "xk", ci, nc.sync)
                stage1(Bp, "xv", ci, nc.sync)
            for ci in range(len(Bp["qch"])):
                stage1(Bp, "xq", ci, nc.sync)

        em = _Emitter(nc, P, wts)

        # -------- phase A flow --------
        em.kproj_chunk(A, 0)
        em.qproj_chunk(A, 0)
        rest = deque(_phase_units(em, Bp)) if Bp is not None else deque()
        for ci in range(nqA):
            if ci == 1 and Bp is not None:
                stage_B()
            if ci == 0:
                due = _chunk0_due(em, A)
                anytime = deque()
            else:
                due = deque()
                if ci + 1 < nqA:
                    due.append((max(0, A["NK"] - 3),
                                lambda ci=ci: em.qproj_chunk(A, ci + 1)))
                # B's fillers from chunk 2 on (their DMA lands after A's)
                anytime = rest if ci >= 2 else deque()
            otd = em.ladder(A, ci, due, anytime)
            em.epilogue(A, ci, otd)
            c0, n = A["qch"][ci]
            nc.sync.dma_start(A["io"]["out"][:, :, c0:c0 + n],
                              A["OTs"][:, :, c0:c0 + n])

        # -------- phase B flow --------
        if Bp is not None:
            if Bp["xk_tiles"][0] is None:
                stage_B()
            while rest:
                rest.popleft()()
            if 0 not in Bp["qcs"]:
                em.qproj_chunk(Bp, 0)
            for ci in range(len(Bp["qch"])):
                otd = em.ladder(Bp, ci, deque(), deque())
                if ci + 1 < len(Bp["qch"]):
                    em.qproj_chunk(Bp, ci + 1)
                em.epilogue(Bp, ci, otd)
                c0, n = Bp["qch"][ci]
                nc.sync.dma_start(Bp["io"]["out"][:, :, c0:c0 + n],
                                  Bp["OTs"][:, :, c0:c0 + n])

    nc.compile()
    return nc


def _prep_xT(X, Pq):
    """[T, D] -> [128, KCH, Pq] bf16 with x[p, k, t] = X[t, k*128 + p]."""
    Xp = np.ascontiguousarray(X[:Pq].T)                 # [D, Pq]
    return np.ascontiguousarray(
        Xp.reshape(KCH, 128, Pq).transpose(1, 0, 2)).astype(BNP)


def _prep_w(W, c):
    """[D, H*DH] -> per-core [128, KCH, 128] bf16 slice of heads (2c, 2c+1)."""
    Ws = W[:, c * 128:(c + 1) * 128]                    # [D, 128]
    return np.ascontiguousarray(
        Ws.reshape(KCH, 128, 128).transpose(1, 0, 2)).astype(BNP)


def kernel(Q_seq, K_seq, V_seq, Q_len, V_len, WQ, WK, WV):
    global LAST_EXEC_NS
    Q_seq = np.asarray(Q_seq, dtype=np.float32)
    K_seq = np.asarray(K_seq, dtype=np.float32)
    V_seq = np.asarray(V_seq, dtype=np.float32)
    WQ = np.asarray(WQ, dtype=np.float32)
    WK = np.asarray(WK, dtype=np.float32)
    WV = np.asarray(WV, dtype=np.float32)
    qlen = [int(np.asarray(Q_len)[b, 0]) for b in range(B)]
    vlen = [int(np.asarray(V_len)[b, 0]) for b in range(B)]

    phases = []
    for b in range(B):
        Qp = _ceil_div(qlen[b], 32) * 32   # q only needs 32-elem alignment
        if Qp == 0:
            continue  # whole batch output is zero
        if vlen[b] > 0:
            NK, scale = _ceil_div(vlen[b], 128), SCALE
        else:
            # all keys masked -> reference softmax degenerates to uniform
            # over all T keys; exp(0*S + 0) = 1 reproduces it exactly.
            NK, scale = T // 128, 0.0
        phases.append(dict(b=b, NK=NK, Qp=Qp, Kp=NK * 128, scale=scale))
    phases.sort(key=lambda ph: -ph["Qp"])  # big phase first (filler donor)

    out = np.zeros((B, T, H * DH), dtype=np.float32)
    if not phases:
        return out

    nc = _build_program(phases)

    # per-phase data shared by all cores
    shared = {}
    for ph in phases:
        b, s, Qp, Kp, NK = ph["b"], str(ph["b"]), ph["Qp"], ph["Kp"], ph["NK"]
        kbias = np.where(np.arange(Kp) < vlen[b], 0.0,
                         -NEG_BIG if vlen[b] > 0 else 0.0)
        kbias = np.ascontiguousarray(
            kbias.astype(np.float32).reshape(NK, 128).T)        # [128, NK]
        d = {"kb" + s: kbias}
        for key, X, Pq in (("xq", Q_seq[b], Qp), ("xk", K_seq[b], Kp),
                           ("xv", V_seq[b], Kp)):
            full = _prep_xT(X, Pq)                              # [128, KCH, Pq]
            for ci, (c0, n) in enumerate(_chunks(Pq)):
                d[f"{key}{s}c{ci}"] = np.ascontiguousarray(
                    full[:, :, c0:c0 + n])
        shared[s] = d

    in_maps = []
    for c in range(N_CORES):
        m = {}
        for ph in phases:
            m.update(shared[str(ph["b"])])
        m["wq"] = _prep_w(WQ, c)
        m["wk"] = _prep_w(WK, c)
        m["wv"] = _prep_w(WV, c)
        in_maps.append(m)

    trace = bool(os.environ.get("BASS_TRACE"))
    if trace:
        _ensure_ntff_hook()
    res = run_bass_kernel_spmd(nc, in_maps, list(range(N_CORES)), trace=trace)
    LAST_EXEC_NS = res.exec_time_ns

    for c in range(N_CORES):
        r = res.results[c]
        for ph in phases:
            b, s, ql = ph["b"], str(ph["b"]), qlen[ph["b"]]
            o = np.asarray(r["out" + s]).astype(np.float32)  # [64, 2, Qp]
            for h in (0, 1):
                head = 2 * c + h
                out[b, :ql, head * DH:(head + 1) * DH] = o[:, h, :ql].T
    return out


# revision 37
# speedup vs baseline: 1.3850x; 1.0146x over previous
"""Trainium2 Bass kernel: masked multi-head attention, sharded across 8 NeuronCores.

Problem shapes (hardcoded): B=2, T=2048, D=1024, H=16 heads, dh=64.

Sharding: one SPMD program with two phases (one per batch element). In each
phase every core handles 2 of the 16 heads (core c -> heads 2c, 2c+1), so the
16 heads of each batch are spread over all 8 cores. This load-balances the
data-dependent work (Q_len/V_len trim the q/k tile counts per batch).

v2 changes vs the fp32 baseline:
  - bf16 inputs/weights/intermediates: matmuls run at 1 cycle/row instead of
    fp32's 4 (fp32 lowers to 2 half-speed passes on TRN2), DMA bytes halve.
  - The two heads' S^T matmuls (K=64 each) are row-tiled to disjoint PE
    quadrants (tile_position (0,0)/(64,0)) so they execute concurrently.
  - exp() for both heads merged into one ACT instruction over a 2-bank PSUM
    tile [128, 2, n] (ACT is the #2 engine; fewer/larger instrs).
  - Epilogue: numerator copied once (DVE), softmax denominator row pulled out
    of PSUM by a tiny DMA, reciprocal_approx_fast on DVE (the old
    single-lane RECIPROCAL was 2.2us/chunk), broadcast over partitions with a
    K=1 f32r matmul, one fused multiply per head.
  - Query-length masking moved to the host gather (rows >= Q_len are simply
    not copied out; the output buffer is pre-zeroed) - no qmask work on HW.
  - The second batch's projections are emitted as filler units inside the
    first batch's ACT-paced attention ladder to keep the PE busy.
"""

import math
import os
from collections import deque
from contextlib import ExitStack

import numpy as np
import ml_dtypes

import concourse.bacc as bacc
import concourse.mybir as mybir
import concourse.tile as tile
from concourse.bass_utils import run_bass_kernel_spmd

F32 = mybir.dt.float32
F32R = mybir.dt.float32r
BF16 = mybir.dt.bfloat16
EXP = mybir.ActivationFunctionType.Exp
BNP = ml_dtypes.bfloat16

B, T, D, H, DH = 2, 2048, 1024, 16, 64
N_CORES = 8
KCH = D // 128          # 8 contraction chunks of the model dim
NEG_BIG = 1.0e12
SCALE = 1.0 / math.sqrt(DH)

LAST_EXEC_NS = None     # filled when BASS_TRACE=1


def _ensure_ntff_hook():
    """run_bass_kernel_spmd(trace=True) imports antenv.axon_hooks, which some
    containers lack; synthesize it (backed by libaxon_pjrt's NRT profiling)
    so tracing degrades gracefully instead of crashing."""
    import sys
    import types
    try:
        import antenv.axon_hooks  # noqa: F401
        return
    except ImportError:
        pass
    try:
        import antenv
        from trn_agent_boot.trn_boot import _ntff_profile_via_ctypes
        hook = _ntff_profile_via_ctypes("/opt/axon/libaxon_pjrt.so")
    except Exception:
        antenv = None
        hook = None
    try:
        m = types.ModuleType("antenv.axon_hooks")
        m._hook = hook
        m.set_axon_ntff_profile_hook = lambda h: setattr(m, "_hook", h)
        m.get_axon_ntff_profile_hook = lambda: m._hook
        sys.modules["antenv.axon_hooks"] = m
        if antenv is not None:
            antenv.axon_hooks = m
    except Exception:
        pass


def _ceil_div(a, b):
    return -(-a // b)


def _chunks(total, w=512):
    out = []
    c = 0
    while c < total:
        out.append((c, min(w, total - c)))
        c += w
    return out


class _Emitter:
    def __init__(self, nc, P, wts):
        self.nc = nc
        self.P = P
        self.wts = wts

    # ---------- projection units (each returns nothing, emits instrs) ------

    def kproj_chunk(self, ph, ci, kr=(0, KCH), st=None):
        """Project keys chunk ci: kc[:, c0:c0+n] = (WK.T @ K_seq.T) slice.
        `kr` bounds the contraction range so a chunk can be emitted as two
        filler halves sharing the psum tile passed via `st`."""
        nc, P = self.nc, self.P
        c0, n = ph["kch"][ci]
        xt = ph["xk_tiles"][ci]
        if kr[0] == 0:
            ps = P["pp"].tile([128, 512], F32, tag="pp", name="kps")
            if st is not None:
                st["ps"] = ps
        else:
            ps = st["ps"]
        for k in range(*kr):
            nc.tensor.matmul(ps[:, :n], lhsT=self.wts["wk"][:, k, :],
                             rhs=xt[:, k, :n],
                             start=(k == 0), stop=(k == KCH - 1),
                             skip_group_check=True)
        if kr[1] == KCH:
            nc.vector.tensor_copy(ph["kc"][:, c0:c0 + n], ps[:, :n])

    def vproj_tile(self, ph, m):
        """Project value tokens [m*128,(m+1)*128) into va[:, m, :, 0:64]."""
        nc, P = self.nc, self.P
        ci, r = divmod(m * 128, 512)
        c0, cn = ph["vch"][ci]
        xt = ph["xv_tiles"][ci]
        ps = P["pp"].tile([128, 512], F32, tag="pp", name="vps")
        for k in range(KCH):
            nc.tensor.matmul(ps[:, 0:128], lhsT=xt[:, k, r:r + 128],
                             rhs=self.wts["wv"][:, k, :],
                             start=(k == 0), stop=(k == KCH - 1),
                             skip_group_check=True)
        nc.vector.tensor_copy(
            ph["va"][:, m, :, 0:64],
            ps[:, 0:128].rearrange("p (g d) -> p g d", g=2))

    def qproj_chunk(self, ph, ci):
        """Project queries chunk ci into the qc ring; returns the tile."""
        nc, P = self.nc, self.P
        c0, n = ph["qch"][ci]
        xt = ph["xq_tiles"][ci]
        ps = P["pp"].tile([128, 512], F32, tag="pp", name="qps")
        for k in range(KCH):
            nc.tensor.matmul(ps[:, :n], lhsT=self.wts["wq"][:, k, :],
                             rhs=xt[:, k, :n],
                             start=(k == 0), stop=(k == KCH - 1),
                             skip_group_check=True)
        qc = P["qc"].tile([128, 512], BF16, tag="qc" + str(ph["b"]),
                          name="qc", bufs=3)
        nc.vector.tensor_copy(qc[:, :n], ps[:, :n])
        ph["qcs"][ci] = qc
        return qc

    # ---------- attention ladder ------------------------------------------

    def ladder(self, ph, ci, due, anytime):
        """S/exp/PV software pipeline for q chunk ci.

        `due`: deque of (deadline_step, closure) in non-decreasing deadline
        order — every unit whose deadline has arrived is emitted that step
        (these carry dataflow deadlines, e.g. vproj(kt) before PV(kt)).
        `anytime`: deque of independent filler closures; at most one is
        popped per step, only on steps with no due unit (keeps PE work per
        step under the ACT exp cadence)."""
        nc, P = self.nc, self.P
        c0, n = ph["qch"][ci]
        NK = ph["NK"]
        qc = ph["qcs"].pop(ci)
        kb = ph["kb_tile"]
        kc, va = ph["kc"], ph["va"]
        scale = ph["scale"]

        otd = P["ot"].tile([65, 2, 512], F32, tag="ot", name="otd")

        def emit_s(kt):
            sps = P["sp"].tile([128, 2, 512], F32, tag="sp", name="sps")
            for h in (0, 1):
                nc.tensor.matmul(
                    sps[:, h, :n],
                    lhsT=kc[h * 64:(h + 1) * 64, kt * 128:(kt + 1) * 128],
                    rhs=qc[h * 64:(h + 1) * 64, :n],
                    start=True, stop=True,
                    tile_position=(h * 64, 0),
                    skip_group_check=True)
            e = P["e"].tile([128, 2, 512], BF16, tag="e", name="e", bufs=3)
            nc.scalar.activation(e[:, :, :n], sps[:, :, :n], EXP,
                                 bias=kb[:, kt:kt + 1], scale=scale)
            return e

        ep = emit_s(0)
        for kt in range(NK):
            ec = ep
            if kt + 1 < NK:
                ep = emit_s(kt + 1)
            popped = False
            while due and due[0][0] <= kt:
                due.popleft()[1]()
                popped = True
            if not popped and anytime:
                anytime.popleft()()
            for h in (0, 1):
                nc.tensor.matmul(otd[:, h, :n], lhsT=va[:, kt, h, :],
                                 rhs=ec[:, h, :n],
                                 start=(kt == 0), stop=(kt == NK - 1),
                                 skip_group_check=True)
        return otd

    def epilogue(self, ph, ci, otd):
        """Normalize otd -> OTs[:, :, c0:c0+n] (no qmask: host trims)."""
        nc, P = self.nc, self.P
        c0, n = ph["qch"][ci]
        ou = P["ou"].tile([64, 2, 512], BF16, tag="ou", name="ou", bufs=2)
        nc.vector.tensor_copy(ou[:, :, :n], otd[0:64, :, :n])
        drow = P["rows"].tile([65, 2, 512], BF16, tag="drow", name="drow",
                              bufs=2)
        nc.vector.tensor_copy(drow[64:65, :, :n], otd[64:65, :, :n])
        rsb = P["rows"].tile([64, 2, 512], F32, tag="rsb", name="rsb",
                             bufs=2)
        for h in (0, 1):
            # broadcast d over 64 partitions (K=1 bf16 matmul), then
            # reciprocal on the [64, n] block (DVE cost is free-size-based,
            # so this is no dearer than a single-partition reciprocal).
            dps = P["pp"].tile([128, 512], F32, tag="pp", name="dps")
            nc.tensor.matmul(dps[0:64, :n],
                             lhsT=P["onesr"][64:65, 0:64],
                             rhs=drow[64:65, h, :n],
                             start=True, stop=True, skip_group_check=True)
            nc.vector.reciprocal_approx_fast(rsb[:, h, :n], dps[0:64, :n])
            nc.vector.tensor_mul(ph["OTs"][:, h, c0:c0 + n],
                                 ou[:, h, :n], rsb[:, h, :n])


def _chunk0_due(em, ph):
    """Deadline units for the first q chunk's ladder: the phase's remaining
    k-proj chunks and all v-proj tiles, interleaved in the exact order their
    DMA chunks arrive, plus qproj(1). Deadlines: vproj(m) before PV(m)
    (popped a step early so the DVE copy hides), kproj(ci) before S(4ci)
    which is emitted at step 4ci-1, qproj(1) a few steps before chunk end."""
    NK = ph["NK"]
    due = []
    for m in range(NK):
        if m >= 1 and m % 4 == 0:
            due.append((max(0, m - 3),
                        lambda ci=m // 4: em.kproj_chunk(ph, ci)))
        due.append((max(0, m - 1), lambda m=m: em.vproj_tile(ph, m)))
    if len(ph["qch"]) > 1:
        due.append((max(0, NK - 3), lambda: em.qproj_chunk(ph, 1)))
    due.sort(key=lambda u: u[0])   # stable: ties keep DMA-arrival order
    return deque(due)


def _phase_units(em, ph):
    """Independent filler closures projecting all of phase `ph`'s inputs,
    in DMA-arrival order. kproj chunks are split in two halves so a single
    pop stays under the ladder's per-step PE budget."""
    units = []
    for ci in range(len(ph["kch"])):
        st = {}
        units.append(lambda ci=ci, st=st:
                     em.kproj_chunk(ph, ci, kr=(0, KCH // 2), st=st))
        units.append(lambda ci=ci, st=st:
                     em.kproj_chunk(ph, ci, kr=(KCH // 2, KCH), st=st))
        for m in range(ci * 4, min((ci + 1) * 4, ph["NK"])):
            units.append(lambda m=m: em.vproj_tile(ph, m))
    units.append(lambda: em.qproj_chunk(ph, 0))
    return units


def _build_program(phases):
    nc = bacc.Bacc("TRN2", target_bir_lowering=False, debug=False,
                   num_devices=N_CORES)
    for ph in phases:
        s = str(ph["b"])
        Qp, Kp, NK = ph["Qp"], ph["Kp"], ph["NK"]
        ph["qch"] = _chunks(Qp)
        ph["kch"] = _chunks(Kp)
        ph["vch"] = ph["kch"]
        ph["qcs"] = {}
        io = {
            "kb": nc.dram_tensor("kb" + s, [128, NK], F32, kind="ExternalInput"),
            "out": nc.dram_tensor("out" + s, [64, 2, Qp], BF16, kind="ExternalOutput"),
        }
        # per-chunk input tensors: per-partition-contiguous so each DMA
        # lowers to 128 large descriptors instead of 1KB-strided fragments
        for key, chl in (("xq", ph["qch"]), ("xk", ph["kch"]), ("xv", ph["vch"])):
            for ci, (c0, n) in enumerate(chl):
                io[f"{key}c{ci}"] = nc.dram_tensor(
                    f"{key}{s}c{ci}", [128, KCH, n], BF16, kind="ExternalInput")
        ph["io"] = io

    with tile.TileContext(nc) as tc, ExitStack() as ctx:
        P = {
            "w": ctx.enter_context(tc.tile_pool(name="w", bufs=1)),
            "x": ctx.enter_context(tc.tile_pool(name="x", bufs=1)),
            "xb": ctx.enter_context(tc.tile_pool(name="xb", bufs=1)),
            "qc": ctx.enter_context(tc.tile_pool(name="qc", bufs=3)),
            "e": ctx.enter_context(tc.tile_pool(name="e", bufs=3)),
            "ou": ctx.enter_context(tc.tile_pool(name="ou", bufs=2)),
            "rows": ctx.enter_context(tc.tile_pool(name="rows", bufs=2)),
            "persist": ctx.enter_context(tc.tile_pool(name="persist", bufs=1)),
            "pp": ctx.enter_context(tc.tile_pool(name="pp", bufs=2, space="PSUM")),
            "sp": ctx.enter_context(tc.tile_pool(name="sp", bufs=2, space="PSUM")),
            "ot": ctx.enter_context(tc.tile_pool(name="ot", bufs=1, space="PSUM")),
        }
        onesr = P["w"].tile([65, 64], BF16, tag="onesr", name="onesr")
        nc.vector.memset(onesr[64:65, :], 1.0)
        P["onesr"] = onesr
        warm = P["w"].tile([1, 1], F32, tag="actwarm", name="actwarm")
        nc.vector.memset(warm[:], 0.0)
        nc.scalar.activation(warm[:], warm[:], EXP)

        # PE p-state warmup: dummy bf16 matmuls on zeroed tiles keep the PE
        # clocking up while the first input DMAs land.
        zw = P["w"].tile([128, 128], BF16, tag="zw", name="zw")
        nc.gpsimd.memset(zw[:], 0.0)
        zw2 = P["w"].tile([128, 512], BF16, tag="zw2", name="zw2")
        nc.gpsimd.memset(zw2[:], 0.0)
        for _ in range(2):
            wps = P["sp"].tile([128, 2, 512], F32, tag="sp", name="wps")
            for r in range(4):
                nc.tensor.matmul(wps[:, 0, :], lhsT=zw[:], rhs=zw2[:],
                                 start=(r == 0), stop=(r == 3),
                                 skip_group_check=True)

        # -------- weights --------
        wts = {}
        for nm in ("wk", "wq", "wv"):
            wts[nm] = nc.dram_tensor(nm, [128, KCH, 128], BF16,
                                     kind="ExternalInput")

        def load_w(nm):
            t = P["w"].tile([128, KCH, 128], BF16, tag=nm, name=nm)
            nc.sync.dma_start(t[:], wts[nm][:])
            wts[nm] = t

        # -------- input staging (issue order == consumption order) --------
        A = phases[0]
        Bp = phases[1] if len(phases) > 1 else None
        for ph in phases:
            for key, chl in (("xq", ph["qch"]), ("xk", ph["kch"]),
                             ("xv", ph["vch"])):
                ph[f"{key}_tiles"] = [None] * len(chl)

        def load_kb(ph):
            s = str(ph["b"])
            kb = P["w"].tile([128, ph["NK"]], F32, tag="kb" + s, name="kb")
            nc.sync.dma_start(kb[:], ph["io"]["kb"][:])
            ph["kb_tile"] = kb

        def stage1(ph, key, ci, eng, halves=1):
            """One input chunk -> SBUF, issued from `eng` (DMA trigger issue
            is ~0.6us+size serial per issuing sequencer, so spread engines)."""
            s = str(ph["b"])
            n = dict(xq=ph["qch"], xk=ph["kch"], xv=ph["vch"])[key][ci][1]
            pool = P["x"] if ph is A else P["xb"]
            xt = pool.tile([128, KCH, n], BF16, tag=f"{key}{s}c{ci}",
                           name=f"{key}{s}c{ci}", bufs=1)
            src = ph["io"][f"{key}c{ci}"]
            step = KCH // halves
            for k in range(0, KCH, step):
                eng.dma_start(xt[:, k:k + step, :], src[:, k:k + step, :])
            ph[f"{key}_tiles"][ci] = xt

        # -------- persistent per-phase tiles --------
        for ph in phases:
            s = str(ph["b"])
            ph["kc"] = P["persist"].tile([128, ph["Kp"]], BF16,
                                         tag="kc" + s, name="kc" + s)
            ph["va"] = P["persist"].tile([128, ph["NK"], 2, 65], BF16,
                                         tag="va" + s, name="va" + s)
            nc.gpsimd.memset(ph["va"][:, :, :, 64:65], 1.0)
            ph["OTs"] = P["persist"].tile([64, 2, ph["Qp"]], BF16,
                                          tag="oT" + s, name="oT" + s)

        # All of phase A's DMA is issued serially from SP in exact
        # consumption order: issue order is the only priority mechanism the
        # 16 shared queues honor, and ring backpressure then throttles SP
        # naturally. Phase B is staged later (inside the chunk-1 emission)
        # on GpSimd software-DGE behind a dependency gate.
        nkA, nqA = len(A["kch"]), len(A["qch"])
        load_w("wk")
        stage1(A, "xk", 0, nc.sync, halves=2)
        load_w("wq")
        stage1(A, "xq", 0, nc.sync)
        load_kb(A)
        if Bp is not None:
            load_kb(Bp)
        load_w("wv")
        stage1(A, "xv", 0, nc.sync)
        for ci in range(1, nkA):
            stage1(A, "xk", ci, nc.sync)
            if ci == nkA - 1 and nqA > 1:
                stage1(A, "xq", 1, nc.sync)
            stage1(A, "xv", ci, nc.sync)
        if nkA == 1 and nqA > 1:
            stage1(A, "xq", 1, nc.sync)
        for ci in range(2, nqA):
            stage1(A, "xq", ci, nc.sync)

        def stage_B():
            # SP's serial trigger FIFO is the throttle: these fire only
            # after all of phase A's transfers have been enqueued
            for ci in range(len(Bp["kch"])):
                stage1(Bp, "xk", ci, nc.sync)
                stage1(Bp, "xv", ci, nc.sync)
            for ci in range(len(Bp["qch"])):
                stage1(Bp, "xq", ci, nc.sync)

        em = _Emitter(nc, P, wts)

        # -------- phase A flow --------
        em.kproj_chunk(A, 0)
        em.qproj_chunk(A, 0)
        rest = deque(_phase_units(em, Bp)) if Bp is not None else deque()
        for ci in range(nqA):
            if ci == 1 and Bp is not None:
                stage_B()
            if ci == 0:
                due = _chunk0_due(em, A)
                anytime = deque()
            else:
                due = deque()
                if ci + 1 < nqA:
                    due.append((max(0, A["NK"] - 3),
                                lambda ci=ci: em.qproj_chunk(A, ci + 1)))
                # B's fillers from chunk 2 on (their DMA lands after A's)
                anytime = rest if ci >= 2 else deque()
            otd = em.ladder(A, ci, due, anytime)
            em.epilogue(A, ci, otd)
            c0, n = A["qch"][ci]
            nc.gpsimd.dma_start(A["io"]["out"][:, :, c0:c0 + n],
                                A["OTs"][:, :, c0:c0 + n])

        # -------- phase B flow --------
        if Bp is not None:
            if Bp["xk_tiles"][0] is None:
                stage_B()
            while rest:
                rest.popleft()()
            if 0 not in Bp["qcs"]:
                em.qproj_chunk(Bp, 0)
            for ci in range(len(Bp["qch"])):
                otd = em.ladder(Bp, ci, deque(), deque())
                if ci + 1 < len(Bp["qch"]):
                    em.qproj_chunk(Bp, ci + 1)
                em.epilogue(Bp, ci, otd)
                c0, n = Bp["qch"][ci]
                nc.gpsimd.dma_start(Bp["io"]["out"][:, :, c0:c0 + n],
                                    Bp["OTs"][:, :, c0:c0 + n])

    nc.compile()
    return nc


def _prep_xT(X, Pq):
    """[T, D] -> [128, KCH, Pq] bf16 with x[p, k, t] = X[t, k*128 + p]."""
    Xp = np.ascontiguousarray(X[:Pq].T)                 # [D, Pq]
    return np.ascontiguousarray(
        Xp.reshape(KCH, 128, Pq).transpose(1, 0, 2)).astype(BNP)


def _prep_w(W, c):
    """[D, H*DH] -> per-core [128, KCH, 128] bf16 slice of heads (2c, 2c+1)."""
    Ws = W[:, c * 128:(c + 1) * 128]                    # [D, 128]
    return np.ascontiguousarray(
        Ws.reshape(KCH, 128, 128).transpose(1, 0, 2)).astype(BNP)


def kernel(Q_seq, K_seq, V_seq, Q_len, V_len, WQ, WK, WV):
    global LAST_EXEC_NS
    Q_seq = np.asarray(Q_seq, dtype=np.float32)
    K_seq = np.asarray(K_seq, dtype=np.float32)
    V_seq = np.asarray(V_seq, dtype=np.float32)
    WQ = np.asarray(WQ, dtype=np.float32)
    WK = np.asarray(WK, dtype=np.float32)
    WV = np.asarray(WV, dtype=np.float32)
    qlen = [int(np.asarray(Q_len)[b, 0]) for b in range(B)]
    vlen = [int(np.asarray(V_len)[b, 0]) for b in range(B)]

    phases = []
    for b in range(B):
        Qp = _ceil_div(qlen[b], 32) * 32   # q only needs 32-elem alignment
        if Qp == 0:
            continue  # whole batch output is zero
        if vlen[b] > 0:
            NK, scale = _ceil_div(vlen[b], 128), SCALE
        else:
            # all keys masked -> reference softmax degenerates to uniform
            # over all T keys; exp(0*S + 0) = 1 reproduces it exactly.
            NK, scale = T // 128, 0.0
        phases.append(dict(b=b, NK=NK, Qp=Qp, Kp=NK * 128, scale=scale))
    phases.sort(key=lambda ph: -ph["Qp"])  # big phase first (filler donor)

    out = np.zeros((B, T, H * DH), dtype=np.float32)
    if not phases:
        return out

    nc = _build_program(phases)

    # per-phase data shared by all cores
    shared = {}
    for ph in phases:
        b, s, Qp, Kp, NK = ph["b"], str(ph["b"]), ph["Qp"], ph["Kp"], ph["NK"]
        kbias = np.where(np.arange(Kp) < vlen[b], 0.0,
                         -NEG_BIG if vlen[b] > 0 else 0.0)
        kbias = np.ascontiguousarray(
            kbias.astype(np.float32).reshape(NK, 128).T)        # [128, NK]
        d = {"kb" + s: kbias}
        for key, X, Pq in (("xq", Q_seq[b], Qp), ("xk", K_seq[b], Kp),
                           ("xv", V_seq[b], Kp)):
            full = _prep_xT(X, Pq)                              # [128, KCH, Pq]
            for ci, (c0, n) in enumerate(_chunks(Pq)):
                d[f"{key}{s}c{ci}"] = np.ascontiguousarray(
                    full[:, :, c0:c0 + n])
        shared[s] = d

    in_maps = []
    for c in range(N_CORES):
        m = {}
        for ph in phases:
            m.update(shared[str(ph["b"])])
        m["wq"] = _prep_w(WQ, c)
        m["wk"] = _prep_w(WK, c)
        m["wv"] = _prep_w(WV, c)
        in_maps.append(m)

    trace = bool(os.environ.get("BASS_TRACE"))
    if trace:
        _ensure_ntff_hook()
    res = run_bass_kernel_spmd(nc, in_maps, list(range(N_CORES)), trace=trace)
    LAST_EXEC_NS = res.exec_time_ns

    for c in range(N_CORES):
        r = res.results[c]
        for ph in phases:
            b, s, ql = ph["b"], str(ph["b"]), qlen[ph["b"]]
            o = np.asarray(r["out" + s]).astype(np.float32)  # [64, 2, Qp]
            for h in (0, 1):
                head = 2 * c + h
                out[b, :ql, head * DH:(head + 1) * DH] = o[:, h, :ql].T
    return out


# revision 40
# speedup vs baseline: 1.3987x; 1.0099x over previous
"""Trainium2 Bass kernel: masked multi-head attention, sharded across 8 NeuronCores.

Problem shapes (hardcoded): B=2, T=2048, D=1024, H=16 heads, dh=64.

Sharding: one SPMD program with two phases (one per batch element). In each
phase every core handles 2 of the 16 heads (core c -> heads 2c, 2c+1), so the
16 heads of each batch are spread over all 8 cores. This load-balances the
data-dependent work (Q_len/V_len trim the q/k tile counts per batch).

v2 changes vs the fp32 baseline:
  - bf16 inputs/weights/intermediates: matmuls run at 1 cycle/row instead of
    fp32's 4 (fp32 lowers to 2 half-speed passes on TRN2), DMA bytes halve.
  - The two heads' S^T matmuls (K=64 each) are row-tiled to disjoint PE
    quadrants (tile_position (0,0)/(64,0)) so they execute concurrently.
  - exp() for both heads merged into one ACT instruction over a 2-bank PSUM
    tile [128, 2, n] (ACT is the #2 engine; fewer/larger instrs).
  - Epilogue: numerator copied once (DVE), softmax denominator row pulled out
    of PSUM by a tiny DMA, reciprocal_approx_fast on DVE (the old
    single-lane RECIPROCAL was 2.2us/chunk), broadcast over partitions with a
    K=1 f32r matmul, one fused multiply per head.
  - Query-length masking moved to the host gather (rows >= Q_len are simply
    not copied out; the output buffer is pre-zeroed) - no qmask work on HW.
  - The second batch's projections are emitted as filler units inside the
    first batch's ACT-paced attention ladder to keep the PE busy.
"""

import math
import os
from collections import deque
from contextlib import ExitStack

import numpy as np
import ml_dtypes

import concourse.bacc as bacc
import concourse.mybir as mybir
import concourse.tile as tile
from concourse.bass_utils import run_bass_kernel_spmd

F32 = mybir.dt.float32
F32R = mybir.dt.float32r
BF16 = mybir.dt.bfloat16
EXP = mybir.ActivationFunctionType.Exp
BNP = ml_dtypes.bfloat16

B, T, D, H, DH = 2, 2048, 1024, 16, 64
N_CORES = 8
KCH = D // 128          # 8 contraction chunks of the model dim
NEG_BIG = 1.0e12
SCALE = 1.0 / math.sqrt(DH)

LAST_EXEC_NS = None     # filled when BASS_TRACE=1


def _ensure_ntff_hook():
    """run_bass_kernel_spmd(trace=True) imports antenv.axon_hooks, which some
    containers lack; synthesize it (backed by libaxon_pjrt's NRT profiling)
    so tracing degrades gracefully instead of crashing."""
    import sys
    import types
    try:
        import antenv.axon_hooks  # noqa: F401
        return
    except ImportError:
        pass
    try:
        import antenv
        from trn_agent_boot.trn_boot import _ntff_profile_via_ctypes
        hook = _ntff_profile_via_ctypes("/opt/axon/libaxon_pjrt.so")
    except Exception:
        antenv = None
        hook = None
    try:
        m = types.ModuleType("antenv.axon_hooks")
        m._hook = hook
        m.set_axon_ntff_profile_hook = lambda h: setattr(m, "_hook", h)
        m.get_axon_ntff_profile_hook = lambda: m._hook
        sys.modules["antenv.axon_hooks"] = m
        if antenv is not None:
            antenv.axon_hooks = m
    except Exception:
        pass


def _ceil_div(a, b):
    return -(-a // b)


def _chunks(total, w=512):
    out = []
    c = 0
    while c < total:
        out.append((c, min(w, total - c)))
        c += w
    return out


class _Emitter:
    def __init__(self, nc, P, wts):
        self.nc = nc
        self.P = P
        self.wts = wts

    # ---------- projection units (each returns nothing, emits instrs) ------

    def kproj_chunk(self, ph, ci, kr=(0, KCH), st=None):
        """Project keys chunk ci: kc[:, c0:c0+n] = (WK.T @ K_seq.T) slice.
        `kr` bounds the contraction range so a chunk can be emitted as two
        filler halves sharing the psum tile passed via `st`."""
        nc, P = self.nc, self.P
        c0, n = ph["kch"][ci]
        xt = ph["xk_tiles"][ci]
        if kr[0] == 0:
            ps = P["pp"].tile([128, 512], F32, tag="pp", name="kps")
            if st is not None:
                st["ps"] = ps
        else:
            ps = st["ps"]
        for k in range(*kr):
            nc.tensor.matmul(ps[:, :n], lhsT=self.wts["wk"][:, k, :],
                             rhs=xt[:, k, :n],
                             start=(k == 0), stop=(k == KCH - 1),
                             skip_group_check=True)
        if kr[1] == KCH:
            nc.vector.tensor_copy(ph["kc"][:, c0:c0 + n], ps[:, :n])

    def vproj_tile(self, ph, m):
        """Project value tokens [m*128,(m+1)*128) into va[:, m, :, 0:64]."""
        nc, P = self.nc, self.P
        ci, r = divmod(m * 128, 512)
        c0, cn = ph["vch"][ci]
        xt = ph["xv_tiles"][ci]
        ps = P["pp"].tile([128, 512], F32, tag="pp", name="vps")
        for k in range(KCH):
            nc.tensor.matmul(ps[:, 0:128], lhsT=xt[:, k, r:r + 128],
                             rhs=self.wts["wv"][:, k, :],
                             start=(k == 0), stop=(k == KCH - 1),
                             skip_group_check=True)
        nc.vector.tensor_copy(
            ph["va"][:, m, :, 0:64],
            ps[:, 0:128].rearrange("p (g d) -> p g d", g=2))

    def qproj_chunk(self, ph, ci):
        """Project queries chunk ci into the qc ring; returns the tile."""
        nc, P = self.nc, self.P
        c0, n = ph["qch"][ci]
        xt = ph["xq_tiles"][ci]
        ps = P["pp"].tile([128, 512], F32, tag="pp", name="qps")
        for k in range(KCH):
            nc.tensor.matmul(ps[:, :n], lhsT=self.wts["wq"][:, k, :],
                             rhs=xt[:, k, :n],
                             start=(k == 0), stop=(k == KCH - 1),
                             skip_group_check=True)
        qc = P["qc"].tile([128, 512], BF16, tag="qc" + str(ph["b"]),
                          name="qc", bufs=3)
        nc.vector.tensor_copy(qc[:, :n], ps[:, :n])
        ph["qcs"][ci] = qc
        return qc

    # ---------- attention ladder ------------------------------------------

    def ladder(self, ph, ci, due, anytime):
        """S/exp/PV software pipeline for q chunk ci.

        `due`: deque of (deadline_step, closure) in non-decreasing deadline
        order — every unit whose deadline has arrived is emitted that step
        (these carry dataflow deadlines, e.g. vproj(kt) before PV(kt)).
        `anytime`: deque of independent filler closures; at most one is
        popped per step, only on steps with no due unit (keeps PE work per
        step under the ACT exp cadence)."""
        nc, P = self.nc, self.P
        c0, n = ph["qch"][ci]
        NK = ph["NK"]
        qc = ph["qcs"].pop(ci)
        kb = ph["kb_tile"]
        kc, va = ph["kc"], ph["va"]
        scale = ph["scale"]

        otd = P["ot"].tile([65, 2, 512], F32, tag="ot", name="otd")

        def emit_s(kt):
            sps = P["sp"].tile([128, 2, 512], F32, tag="sp", name="sps")
            for h in (0, 1):
                nc.tensor.matmul(
                    sps[:, h, :n],
                    lhsT=kc[h * 64:(h + 1) * 64, kt * 128:(kt + 1) * 128],
                    rhs=qc[h * 64:(h + 1) * 64, :n],
                    start=True, stop=True,
                    tile_position=(h * 64, 0),
                    skip_group_check=True)
            e = P["e"].tile([128, 2, 512], BF16, tag="e", name="e", bufs=4)
            nc.scalar.activation(e[:, :, :n], sps[:, :, :n], EXP,
                                 bias=kb[:, kt:kt + 1], scale=scale)
            return e

        ep = emit_s(0)
        for kt in range(NK):
            ec = ep
            if kt + 1 < NK:
                ep = emit_s(kt + 1)
            popped = False
            while due and due[0][0] <= kt:
                due.popleft()[1]()
                popped = True
            if not popped and anytime:
                anytime.popleft()()
            for h in (0, 1):
                nc.tensor.matmul(otd[:, h, :n], lhsT=va[:, kt, h, :],
                                 rhs=ec[:, h, :n],
                                 start=(kt == 0), stop=(kt == NK - 1),
                                 skip_group_check=True)
        return otd

    def epilogue_release(self, ph, ci, otd):
        """Copy numerator + denominator row out of PSUM (frees otd fast)."""
        nc, P = self.nc, self.P
        c0, n = ph["qch"][ci]
        ou = P["ou"].tile([64, 2, 512], BF16, tag="ou", name="ou", bufs=2)
        nc.vector.tensor_copy(ou[:, :, :n], otd[0:64, :, :n])
        drow = P["rows"].tile([65, 2, 512], BF16, tag="drow", name="drow",
                              bufs=2)
        nc.vector.tensor_copy(drow[64:65, :, :n], otd[64:65, :, :n])
        return ou, drow

    def epilogue_norm(self, ph, ci, ou, drow):
        """Normalize -> OTs slice and DMA it out (no qmask: host trims).
        Deferred into the next ladder via a due-unit so its PE/DVE work
        doesn't block the next chunk's S matmuls in the engine FIFOs."""
        nc, P = self.nc, self.P
        c0, n = ph["qch"][ci]
        rsb = P["rows"].tile([64, 2, 512], F32, tag="rsb", name="rsb",
                             bufs=2)
        for h in (0, 1):
            # broadcast d over 64 partitions (K=1 bf16 matmul), then
            # reciprocal on the [64, n] block (DVE cost is free-size-based,
            # so this is no dearer than a single-partition reciprocal).
            dps = P["pp"].tile([128, 512], F32, tag="pp", name="dps")
            nc.tensor.matmul(dps[0:64, :n],
                             lhsT=P["onesr"][64:65, 0:64],
                             rhs=drow[64:65, h, :n],
                             start=True, stop=True, skip_group_check=True)
            nc.vector.reciprocal_approx_fast(rsb[:, h, :n], dps[0:64, :n])
            nc.vector.tensor_mul(ph["OTs"][:, h, c0:c0 + n],
                                 ou[:, h, :n], rsb[:, h, :n])
        nc.gpsimd.dma_start(ph["io"]["out"][:, :, c0:c0 + n],
                            ph["OTs"][:, :, c0:c0 + n])


def _chunk0_due(em, ph):
    """Deadline units for the first q chunk's ladder: the phase's remaining
    k-proj chunks and all v-proj tiles, interleaved in the exact order their
    DMA chunks arrive, plus qproj(1). Deadlines: vproj(m) before PV(m)
    (popped a step early so the DVE copy hides), kproj(ci) before S(4ci)
    which is emitted at step 4ci-1, qproj(1) a few steps before chunk end."""
    NK = ph["NK"]
    due = []
    for m in range(NK):
        if m >= 1 and m % 4 == 0:
            due.append((max(0, m - 3),
                        lambda ci=m // 4: em.kproj_chunk(ph, ci)))
        due.append((max(0, m - 1), lambda m=m: em.vproj_tile(ph, m)))
    if len(ph["qch"]) > 1:
        due.append((max(0, NK - 3), lambda: em.qproj_chunk(ph, 1)))
    due.sort(key=lambda u: u[0])   # stable: ties keep DMA-arrival order
    return deque(due)


def _phase_units(em, ph):
    """Independent filler closures projecting all of phase `ph`'s inputs,
    in DMA-arrival order. kproj chunks are split in two halves so a single
    pop stays under the ladder's per-step PE budget."""
    units = []
    for ci in range(len(ph["kch"])):
        st = {}
        units.append(lambda ci=ci, st=st:
                     em.kproj_chunk(ph, ci, kr=(0, KCH // 2), st=st))
        units.append(lambda ci=ci, st=st:
                     em.kproj_chunk(ph, ci, kr=(KCH // 2, KCH), st=st))
        for m in range(ci * 4, min((ci + 1) * 4, ph["NK"])):
            units.append(lambda m=m: em.vproj_tile(ph, m))
    units.append(lambda: em.qproj_chunk(ph, 0))
    return units


def _build_program(phases):
    nc = bacc.Bacc("TRN2", target_bir_lowering=False, debug=False,
                   num_devices=N_CORES)
    for ph in phases:
        s = str(ph["b"])
        Qp, Kp, NK = ph["Qp"], ph["Kp"], ph["NK"]
        ph["qch"] = _chunks(Qp)
        ph["kch"] = _chunks(Kp)
        ph["vch"] = ph["kch"]
        ph["qcs"] = {}
        io = {
            "kb": nc.dram_tensor("kb" + s, [128, NK], F32, kind="ExternalInput"),
            "out": nc.dram_tensor("out" + s, [64, 2, Qp], BF16, kind="ExternalOutput"),
        }
        # per-chunk input tensors: per-partition-contiguous so each DMA
        # lowers to 128 large descriptors instead of 1KB-strided fragments
        for key, chl in (("xq", ph["qch"]), ("xk", ph["kch"]), ("xv", ph["vch"])):
            for ci, (c0, n) in enumerate(chl):
                io[f"{key}c{ci}"] = nc.dram_tensor(
                    f"{key}{s}c{ci}", [128, KCH, n], BF16, kind="ExternalInput")
        ph["io"] = io

    with tile.TileContext(nc) as tc, ExitStack() as ctx:
        P = {
            "w": ctx.enter_context(tc.tile_pool(name="w", bufs=1)),
            "x": ctx.enter_context(tc.tile_pool(name="x", bufs=1)),
            "xb": ctx.enter_context(tc.tile_pool(name="xb", bufs=1)),
            "qc": ctx.enter_context(tc.tile_pool(name="qc", bufs=3)),
            "e": ctx.enter_context(tc.tile_pool(name="e", bufs=4)),
            "ou": ctx.enter_context(tc.tile_pool(name="ou", bufs=2)),
            "rows": ctx.enter_context(tc.tile_pool(name="rows", bufs=2)),
            "persist": ctx.enter_context(tc.tile_pool(name="persist", bufs=1)),
            "pp": ctx.enter_context(tc.tile_pool(name="pp", bufs=2, space="PSUM")),
            "sp": ctx.enter_context(tc.tile_pool(name="sp", bufs=2, space="PSUM")),
            "ot": ctx.enter_context(tc.tile_pool(name="ot", bufs=1, space="PSUM")),
        }
        onesr = P["w"].tile([65, 64], BF16, tag="onesr", name="onesr")
        nc.vector.memset(onesr[64:65, :], 1.0)
        P["onesr"] = onesr
        warm = P["w"].tile([1, 1], F32, tag="actwarm", name="actwarm")
        nc.vector.memset(warm[:], 0.0)
        nc.scalar.activation(warm[:], warm[:], EXP)

        # PE p-state warmup: dummy bf16 matmuls on zeroed tiles keep the PE
        # clocking up while the first input DMAs land.
        zw = P["w"].tile([128, 128], BF16, tag="zw", name="zw")
        nc.gpsimd.memset(zw[:], 0.0)
        zw2 = P["w"].tile([128, 512], BF16, tag="zw2", name="zw2")
        nc.gpsimd.memset(zw2[:], 0.0)
        for _ in range(3):
            wps = P["sp"].tile([128, 2, 512], F32, tag="sp", name="wps")
            for r in range(4):
                nc.tensor.matmul(wps[:, 0, :], lhsT=zw[:], rhs=zw2[:],
                                 start=(r == 0), stop=(r == 3),
                                 skip_group_check=True)

        # -------- weights --------
        wts = {}
        for nm in ("wk", "wq", "wv"):
            wts[nm] = nc.dram_tensor(nm, [128, KCH, 128], BF16,
                                     kind="ExternalInput")

        def load_w(nm):
            t = P["w"].tile([128, KCH, 128], BF16, tag=nm, name=nm)
            nc.sync.dma_start(t[:], wts[nm][:])
            wts[nm] = t

        # -------- input staging (issue order == consumption order) --------
        A = phases[0]
        Bp = phases[1] if len(phases) > 1 else None
        for ph in phases:
            for key, chl in (("xq", ph["qch"]), ("xk", ph["kch"]),
                             ("xv", ph["vch"])):
                ph[f"{key}_tiles"] = [None] * len(chl)

        def load_kb(ph):
            s = str(ph["b"])
            kb = P["w"].tile([128, ph["NK"]], F32, tag="kb" + s, name="kb")
            nc.sync.dma_start(kb[:], ph["io"]["kb"][:])
            ph["kb_tile"] = kb

        def stage1(ph, key, ci, eng, halves=1):
            """One input chunk -> SBUF, issued from `eng` (DMA trigger issue
            is ~0.6us+size serial per issuing sequencer, so spread engines)."""
            s = str(ph["b"])
            n = dict(xq=ph["qch"], xk=ph["kch"], xv=ph["vch"])[key][ci][1]
            pool = P["x"] if ph is A else P["xb"]
            xt = pool.tile([128, KCH, n], BF16, tag=f"{key}{s}c{ci}",
                           name=f"{key}{s}c{ci}", bufs=1)
            src = ph["io"][f"{key}c{ci}"]
            step = KCH // halves
            for k in range(0, KCH, step):
                eng.dma_start(xt[:, k:k + step, :], src[:, k:k + step, :])
            ph[f"{key}_tiles"][ci] = xt

        # -------- persistent per-phase tiles --------
        for ph in phases:
            s = str(ph["b"])
            ph["kc"] = P["persist"].tile([128, ph["Kp"]], BF16,
                                         tag="kc" + s, name="kc" + s)
            ph["va"] = P["persist"].tile([128, ph["NK"], 2, 65], BF16,
                                         tag="va" + s, name="va" + s)
            nc.gpsimd.memset(ph["va"][:, :, :, 64:65], 1.0)
            ph["OTs"] = P["persist"].tile([64, 2, ph["Qp"]], BF16,
                                          tag="oT" + s, name="oT" + s)

        # All of phase A's DMA is issued serially from SP in exact
        # consumption order: issue order is the only priority mechanism the
        # 16 shared queues honor, and ring backpressure then throttles SP
        # naturally. Phase B is staged later (inside the chunk-1 emission)
        # on GpSimd software-DGE behind a dependency gate.
        nkA, nqA = len(A["kch"]), len(A["qch"])
        load_w("wk")
        stage1(A, "xk", 0, nc.sync, halves=2)
        load_w("wq")
        stage1(A, "xq", 0, nc.sync)
        load_kb(A)
        if Bp is not None:
            load_kb(Bp)
        load_w("wv")
        stage1(A, "xv", 0, nc.sync)
        for ci in range(1, nkA):
            stage1(A, "xk", ci, nc.sync)
            if ci == nkA - 1 and nqA > 1:
                stage1(A, "xq", 1, nc.sync)
            stage1(A, "xv", ci, nc.sync)
        if nkA == 1 and nqA > 1:
            stage1(A, "xq", 1, nc.sync)
        for ci in range(2, nqA):
            stage1(A, "xq", ci, nc.sync)

        def stage_B():
            # SP's serial trigger FIFO is the throttle: these fire only
            # after all of phase A's transfers have been enqueued
            for ci in range(len(Bp["kch"])):
                stage1(Bp, "xk", ci, nc.sync)
                stage1(Bp, "xv", ci, nc.sync)
            for ci in range(len(Bp["qch"])):
                stage1(Bp, "xq", ci, nc.sync)

        em = _Emitter(nc, P, wts)

        # -------- phase A flow --------
        em.kproj_chunk(A, 0)
        em.qproj_chunk(A, 0)
        rest = deque(_phase_units(em, Bp)) if Bp is not None else deque()
        pending = None
        for ci in range(nqA):
            if ci == 1 and Bp is not None:
                stage_B()
            if ci == 0:
                due = _chunk0_due(em, A)
                anytime = deque()
            else:
                due = deque()
                if pending is not None:
                    due.append((1, pending))
                if ci + 1 < nqA:
                    due.append((max(2, A["NK"] - 3),
                                lambda ci=ci: em.qproj_chunk(A, ci + 1)))
                # B's fillers from chunk 2 on (their DMA lands after A's)
                anytime = rest if ci >= 2 else deque()
            otd = em.ladder(A, ci, due, anytime)
            ou, drow = em.epilogue_release(A, ci, otd)
            pending = (lambda ci=ci, ou=ou, drow=drow:
                       em.epilogue_norm(A, ci, ou, drow))

        # -------- phase B flow --------
        if Bp is not None:
            if Bp["xk_tiles"][0] is None:
                stage_B()
            while rest:
                rest.popleft()()
            if 0 not in Bp["qcs"]:
                em.qproj_chunk(Bp, 0)
            for ci in range(len(Bp["qch"])):
                due = deque()
                if pending is not None:
                    due.append((1, pending))
                otd = em.ladder(Bp, ci, due, deque())
                if ci + 1 < len(Bp["qch"]):
                    em.qproj_chunk(Bp, ci + 1)
                ou, drow = em.epilogue_release(Bp, ci, otd)
                pending = (lambda ci=ci, ou=ou, drow=drow:
                           em.epilogue_norm(Bp, ci, ou, drow))
        if pending is not None:
            pending()

    nc.compile()
    return nc


def _prep_xT(X, Pq):
    """[T, D] -> [128, KCH, Pq] bf16 with x[p, k, t] = X[t, k*128 + p]."""
    Xp = np.ascontiguousarray(X[:Pq].T)                 # [D, Pq]
    return np.ascontiguousarray(
        Xp.reshape(KCH, 128, Pq).transpose(1, 0, 2)).astype(BNP)


def _prep_w(W, c):
    """[D, H*DH] -> per-core [128, KCH, 128] bf16 slice of heads (2c, 2c+1)."""
    Ws = W[:, c * 128:(c + 1) * 128]                    # [D, 128]
    return np.ascontiguousarray(
        Ws.reshape(KCH, 128, 128).transpose(1, 0, 2)).astype(BNP)


def kernel(Q_seq, K_seq, V_seq, Q_len, V_len, WQ, WK, WV):
    global LAST_EXEC_NS
    Q_seq = np.asarray(Q_seq, dtype=np.float32)
    K_seq = np.asarray(K_seq, dtype=np.float32)
    V_seq = np.asarray(V_seq, dtype=np.float32)
    WQ = np.asarray(WQ, dtype=np.float32)
    WK = np.asarray(WK, dtype=np.float32)
    WV = np.asarray(WV, dtype=np.float32)
    qlen = [int(np.asarray(Q_len)[b, 0]) for b in range(B)]
    vlen = [int(np.asarray(V_len)[b, 0]) for b in range(B)]

    phases = []
    for b in range(B):
        Qp = _ceil_div(qlen[b], 32) * 32   # q only needs 32-elem alignment
        if Qp == 0:
            continue  # whole batch output is zero
        if vlen[b] > 0:
            NK, scale = _ceil_div(vlen[b], 128), SCALE
        else:
            # all keys masked -> reference softmax degenerates to uniform
            # over all T keys; exp(0*S + 0) = 1 reproduces it exactly.
            NK, scale = T // 128, 0.0
        phases.append(dict(b=b, NK=NK, Qp=Qp, Kp=NK * 128, scale=scale))
    phases.sort(key=lambda ph: -ph["Qp"])  # big phase first (filler donor)

    out = np.zeros((B, T, H * DH), dtype=np.float32)
    if not phases:
        return out

    nc = _build_program(phases)

    # per-phase data shared by all cores
    shared = {}
    for ph in phases:
        b, s, Qp, Kp, NK = ph["b"], str(ph["b"]), ph["Qp"], ph["Kp"], ph["NK"]
        kbias = np.where(np.arange(Kp) < vlen[b], 0.0,
                         -NEG_BIG if vlen[b] > 0 else 0.0)
        kbias = np.ascontiguousarray(
            kbias.astype(np.float32).reshape(NK, 128).T)        # [128, NK]
        d = {"kb" + s: kbias}
        for key, X, Pq in (("xq", Q_seq[b], Qp), ("xk", K_seq[b], Kp),
                           ("xv", V_seq[b], Kp)):
            full = _prep_xT(X, Pq)                              # [128, KCH, Pq]
            for ci, (c0, n) in enumerate(_chunks(Pq)):
                d[f"{key}{s}c{ci}"] = np.ascontiguousarray(
                    full[:, :, c0:c0 + n])
        shared[s] = d

    in_maps = []
    for c in range(N_CORES):
        m = {}
        for ph in phases:
            m.update(shared[str(ph["b"])])
        m["wq"] = _prep_w(WQ, c)
        m["wk"] = _prep_w(WK, c)
        m["wv"] = _prep_w(WV, c)
        in_maps.append(m)

    trace = bool(os.environ.get("BASS_TRACE"))
    if trace:
        _ensure_ntff_hook()
    res = run_bass_kernel_spmd(nc, in_maps, list(range(N_CORES)), trace=trace)
    LAST_EXEC_NS = res.exec_time_ns

    for c in range(N_CORES):
        r = res.results[c]
        for ph in phases:
            b, s, ql = ph["b"], str(ph["b"]), qlen[ph["b"]]
            o = np.asarray(r["out" + s]).astype(np.float32)  # [64, 2, Qp]
            for h in (0, 1):
                head = 2 * c + h
                out[b, :ql, head * DH:(head + 1) * DH] = o[:, h, :ql].T
    return out


# revision 45
# speedup vs baseline: 1.5101x; 1.0796x over previous
"""Trainium2 Bass kernel: masked multi-head attention, sharded across 8 NeuronCores.

Problem shapes (hardcoded): B=2, T=2048, D=1024, H=16 heads, dh=64.

Sharding: one SPMD program with two phases (one per batch element). In each
phase every core handles 2 of the 16 heads (core c -> heads 2c, 2c+1), so the
16 heads of each batch are spread over all 8 cores. This load-balances the
data-dependent work (Q_len/V_len trim the q/k tile counts per batch).

v2 changes vs the fp32 baseline:
  - bf16 inputs/weights/intermediates: matmuls run at 1 cycle/row instead of
    fp32's 4 (fp32 lowers to 2 half-speed passes on TRN2), DMA bytes halve.
  - The two heads' S^T matmuls (K=64 each) are row-tiled to disjoint PE
    quadrants (tile_position (0,0)/(64,0)) so they execute concurrently.
  - exp() for both heads merged into one ACT instruction over a 2-bank PSUM
    tile [128, 2, n] (ACT is the #2 engine; fewer/larger instrs).
  - Epilogue: numerator copied once (DVE), softmax denominator row pulled out
    of PSUM by a tiny DMA, reciprocal_approx_fast on DVE (the old
    single-lane RECIPROCAL was 2.2us/chunk), broadcast over partitions with a
    K=1 f32r matmul, one fused multiply per head.
  - Query-length masking moved to the host gather (rows >= Q_len are simply
    not copied out; the output buffer is pre-zeroed) - no qmask work on HW.
  - The second batch's projections are emitted as filler units inside the
    first batch's ACT-paced attention ladder to keep the PE busy.
"""

import math
import os
from collections import deque
from contextlib import ExitStack

import numpy as np
import ml_dtypes

import concourse.bacc as bacc
import concourse.mybir as mybir
import concourse.tile as tile
from concourse.bass_utils import run_bass_kernel_spmd

F32 = mybir.dt.float32
F32R = mybir.dt.float32r
BF16 = mybir.dt.bfloat16
EXP = mybir.ActivationFunctionType.Exp
BNP = ml_dtypes.bfloat16

B, T, D, H, DH = 2, 2048, 1024, 16, 64
N_CORES = 8
KCH = D // 128          # 8 contraction chunks of the model dim
NEG_BIG = 1.0e12
SCALE = 1.0 / math.sqrt(DH)

LAST_EXEC_NS = None     # filled when BASS_TRACE=1


def _ensure_ntff_hook():
    """run_bass_kernel_spmd(trace=True) imports antenv.axon_hooks, which some
    containers lack; synthesize it (backed by libaxon_pjrt's NRT profiling)
    so tracing degrades gracefully instead of crashing."""
    import sys
    import types
    try:
        import antenv.axon_hooks  # noqa: F401
        return
    except ImportError:
        pass
    try:
        import antenv
        from trn_agent_boot.trn_boot import _ntff_profile_via_ctypes
        hook = _ntff_profile_via_ctypes("/opt/axon/libaxon_pjrt.so")
    except Exception:
        antenv = None
        hook = None
    try:
        m = types.ModuleType("antenv.axon_hooks")
        m._hook = hook
        m.set_axon_ntff_profile_hook = lambda h: setattr(m, "_hook", h)
        m.get_axon_ntff_profile_hook = lambda: m._hook
        sys.modules["antenv.axon_hooks"] = m
        if antenv is not None:
            antenv.axon_hooks = m
    except Exception:
        pass


def _ceil_div(a, b):
    return -(-a // b)


def _chunks(total, w=512):
    out = []
    c = 0
    while c < total:
        out.append((c, min(w, total - c)))
        c += w
    return out


class _Emitter:
    def __init__(self, nc, P, wts):
        self.nc = nc
        self.P = P
        self.wts = wts

    # ---------- projection units (each returns nothing, emits instrs) ------

    def kproj_chunk(self, ph, ci, kr=(0, KCH), st=None):
        """Project keys chunk ci: kc[:, c0:c0+n] = (WK.T @ K_seq.T) slice.
        `kr` bounds the contraction range so a chunk can be emitted as two
        filler halves sharing the psum tile passed via `st`."""
        nc, P = self.nc, self.P
        c0, n = ph["kch"][ci]
        xt = ph["xk_tiles"][ci]
        if kr[0] == 0:
            ps = P["pp"].tile([128, 512], F32, tag="pp", name="kps")
            if st is not None:
                st["ps"] = ps
        else:
            ps = st["ps"]
        for k in range(*kr):
            nc.tensor.matmul(ps[:, :n], lhsT=self.wts["wk"][:, k, :],
                             rhs=xt[:, k, :n],
                             start=(k == 0), stop=(k == KCH - 1),
                             skip_group_check=True)
        if kr[1] == KCH:
            nc.vector.tensor_copy(ph["kc"][:, c0:c0 + n], ps[:, :n])

    def vproj_tile(self, ph, m):
        """Project value tokens [m*128,(m+1)*128) into va[:, m, :, 0:64]."""
        nc, P = self.nc, self.P
        ci, r = divmod(m * 128, 512)
        c0, cn = ph["vch"][ci]
        xt = ph["xv_tiles"][ci]
        ps = P["pp"].tile([128, 512], F32, tag="pp", name="vps")
        for k in range(KCH):
            nc.tensor.matmul(ps[:, 0:128], lhsT=xt[:, k, r:r + 128],
                             rhs=self.wts["wv"][:, k, :],
                             start=(k == 0), stop=(k == KCH - 1),
                             skip_group_check=True)
        nc.vector.tensor_copy(
            ph["va"][:, m, :, 0:64],
            ps[:, 0:128].rearrange("p (g d) -> p g d", g=2))

    def qproj_chunk(self, ph, ci, kr=(0, KCH), st=None):
        """Project queries chunk ci into the qc ring; returns the tile.
        Like kproj_chunk, can be emitted as two halves via kr/st."""
        nc, P = self.nc, self.P
        c0, n = ph["qch"][ci]
        xt = ph["xq_tiles"][ci]
        if kr[0] == 0:
            ps = P["pp"].tile([128, 512], F32, tag="pp", name="qps")
            if st is not None:
                st["ps"] = ps
        else:
            ps = st["ps"]
        for k in range(*kr):
            nc.tensor.matmul(ps[:, :n], lhsT=self.wts["wq"][:, k, :],
                             rhs=xt[:, k, :n],
                             start=(k == 0), stop=(k == KCH - 1),
                             skip_group_check=True)
        if kr[1] < KCH:
            return None
        qc = P["qc"].tile([128, 512], BF16, tag="qc" + str(ph["b"]),
                          name="qc", bufs=3)
        nc.vector.tensor_copy(qc[:, :n], ps[:, :n])
        ph["qcs"][ci] = qc
        return qc

    # ---------- attention ladder ------------------------------------------

    def ladder(self, ph, ci, due, anytime):
        """S/exp/PV software pipeline for q chunk ci.

        `due`: deque of (deadline_step, closure) in non-decreasing deadline
        order — every unit whose deadline has arrived is emitted that step
        (these carry dataflow deadlines, e.g. vproj(kt) before PV(kt)).
        `anytime`: deque of independent filler closures; at most one is
        popped per step, only on steps with no due unit (keeps PE work per
        step under the ACT exp cadence)."""
        nc, P = self.nc, self.P
        c0, n = ph["qch"][ci]
        NK = ph["NK"]
        qc = ph["qcs"].pop(ci)
        kb = ph["kb_tile"]
        kc, va = ph["kc"], ph["va"]
        scale = ph["scale"]
        nfull = ph["nfull"]    # leading key tiles with all-zero bias

        otd = P["ot"].tile([65, 2, 512], F32, tag="ot", name="otd")

        # group key tiles: zero-bias tiles share one sps tile + ONE exp
        # (per-instruction ACT overhead dominates for narrow chunks)
        cap = max(1, 512 // n)
        groups, kt = [], 0
        while kt < NK:
            g = min(nfull - kt, cap) if kt < nfull else 1
            g = max(g, 1)
            groups.append((kt, g))
            kt += g

        def emit_sg(gi):
            kt0, g = groups[gi]
            sps = P["sp"].tile([128, 2, 512], F32, tag="sp", name="sps")
            for j in range(g):
                for h in (0, 1):
                    kt = kt0 + j
                    nc.tensor.matmul(
                        sps[:, h, j * n:(j + 1) * n],
                        lhsT=kc[h * 64:(h + 1) * 64,
                                kt * 128:(kt + 1) * 128],
                        rhs=qc[h * 64:(h + 1) * 64, :n],
                        start=True, stop=True,
                        tile_position=(h * 64, 0),
                        skip_group_check=True)
            e = P["e"].tile([128, 2, 512], BF16, tag="e", name="e", bufs=4)
            bias = 0.0 if kt0 + g <= nfull else kb[:, kt0:kt0 + 1]
            nc.scalar.activation(e[:, :, :g * n], sps[:, :, :g * n], EXP,
                                 bias=bias, scale=scale)
            return e

        ep = emit_sg(0)
        for gi in range(len(groups)):
            ec = ep
            if gi + 1 < len(groups):
                ep = emit_sg(gi + 1)
            kt0, g = groups[gi]
            popped = 0
            while due and due[0][0] <= kt0 + g - 1:
                due.popleft()[1]()
                popped += 1
            while popped < g and anytime:
                anytime.popleft()()
                popped += 1
            for j in range(g):
                kt = kt0 + j
                for h in (0, 1):
                    nc.tensor.matmul(otd[:, h, :n], lhsT=va[:, kt, h, :],
                                     rhs=ec[:, h, j * n:(j + 1) * n],
                                     start=(kt == 0), stop=(kt == NK - 1),
                                     skip_group_check=True)
        return otd

    def epilogue_release(self, ph, ci, otd):
        """Copy numerator + denominator row out of PSUM (frees otd fast)."""
        nc, P = self.nc, self.P
        c0, n = ph["qch"][ci]
        od = P["ou"].tile([65, 2, 512], BF16, tag="od", name="od", bufs=2)
        nc.vector.tensor_copy(od[:, :, :n], otd[:, :, :n])
        return od

    def epilogue_norm(self, ph, ci, od):
        """Normalize -> OTs slice and DMA it out (no qmask: host trims).
        Deferred into the next ladder via a due-unit so its PE/DVE work
        doesn't block the next chunk's S matmuls in the engine FIFOs."""
        nc, P = self.nc, self.P
        c0, n = ph["qch"][ci]
        rsb = P["rows"].tile([64, 2, 512], F32, tag="rsb", name="rsb",
                             bufs=2)
        for h in (0, 1):
            # broadcast d over 64 partitions (K=1 bf16 matmul), then
            # reciprocal on the [64, n] block (DVE cost is free-size-based,
            # so this is no dearer than a single-partition reciprocal).
            dps = P["pp"].tile([128, 512], F32, tag="pp", name="dps")
            nc.tensor.matmul(dps[0:64, :n],
                             lhsT=P["onesr"][64:65, 0:64],
                             rhs=od[64:65, h, :n],
                             start=True, stop=True, skip_group_check=True)
            nc.vector.reciprocal_approx_fast(rsb[:, h, :n], dps[0:64, :n])
            nc.vector.tensor_mul(ph["OTs"][:, h, c0:c0 + n],
                                 od[0:64, h, :n], rsb[:, h, :n])
        nc.gpsimd.dma_start(ph["io"]["out"][:, :, c0:c0 + n],
                            ph["OTs"][:, :, c0:c0 + n])


def _chunk0_due(em, ph):
    """Deadline units for the first q chunk's ladder: the phase's remaining
    k-proj chunks and all v-proj tiles, interleaved in the exact order their
    DMA chunks arrive, plus qproj(1). Deadlines: vproj(m) before PV(m)
    (popped a step early so the DVE copy hides), kproj(ci) before S(4ci)
    which is emitted at step 4ci-1, qproj(1) a few steps before chunk end."""
    NK = ph["NK"]
    due = []
    for m in range(NK):
        if m >= 1 and m % 4 == 0:
            due.append((max(0, m - 3),
                        lambda ci=m // 4: em.kproj_chunk(ph, ci)))
        due.append((max(0, m - 1), lambda m=m: em.vproj_tile(ph, m)))
    if len(ph["qch"]) > 1:
        due.append((max(0, NK - 3), lambda: em.qproj_chunk(ph, 1)))
    due.sort(key=lambda u: u[0])   # stable: ties keep DMA-arrival order
    return deque(due)


def _phase_units(em, ph):
    """Independent filler closures projecting all of phase `ph`'s inputs,
    in DMA-arrival order. kproj chunks are split in two halves so a single
    pop stays under the ladder's per-step PE budget."""
    units = []
    for ci in range(len(ph["kch"])):
        st = {}
        units.append(lambda ci=ci, st=st:
                     em.kproj_chunk(ph, ci, kr=(0, KCH // 2), st=st))
        units.append(lambda ci=ci, st=st:
                     em.kproj_chunk(ph, ci, kr=(KCH // 2, KCH), st=st))
        for m in range(ci * 4, min((ci + 1) * 4, ph["NK"])):
            units.append(lambda m=m: em.vproj_tile(ph, m))
    units.append(lambda: em.qproj_chunk(ph, 0))
    return units


def _build_program(phases):
    nc = bacc.Bacc("TRN2", target_bir_lowering=False, debug=False,
                   num_devices=N_CORES)
    for ph in phases:
        s = str(ph["b"])
        Qp, Kp, NK = ph["Qp"], ph["Kp"], ph["NK"]
        ph["qch"] = _chunks(Qp)
        ph["kch"] = _chunks(Kp)
        ph["vch"] = ph["kch"]
        ph["qcs"] = {}
        io = {
            "kb": nc.dram_tensor("kb" + s, [128, NK], F32, kind="ExternalInput"),
            "out": nc.dram_tensor("out" + s, [64, 2, Qp], BF16, kind="ExternalOutput"),
        }
        # per-chunk input tensors: per-partition-contiguous so each DMA
        # lowers to 128 large descriptors instead of 1KB-strided fragments
        for key, chl in (("xq", ph["qch"]), ("xk", ph["kch"]), ("xv", ph["vch"])):
            for ci, (c0, n) in enumerate(chl):
                io[f"{key}c{ci}"] = nc.dram_tensor(
                    f"{key}{s}c{ci}", [128, KCH, n], BF16, kind="ExternalInput")
        ph["io"] = io

    with tile.TileContext(nc) as tc, ExitStack() as ctx:
        P = {
            "w": ctx.enter_context(tc.tile_pool(name="w", bufs=1)),
            "x": ctx.enter_context(tc.tile_pool(name="x", bufs=1)),
            "xb": ctx.enter_context(tc.tile_pool(name="xb", bufs=1)),
            "qc": ctx.enter_context(tc.tile_pool(name="qc", bufs=3)),
            "e": ctx.enter_context(tc.tile_pool(name="e", bufs=4)),
            "ou": ctx.enter_context(tc.tile_pool(name="ou", bufs=2)),
            "rows": ctx.enter_context(tc.tile_pool(name="rows", bufs=2)),
            "persist": ctx.enter_context(tc.tile_pool(name="persist", bufs=1)),
            "pp": ctx.enter_context(tc.tile_pool(name="pp", bufs=2, space="PSUM")),
            "sp": ctx.enter_context(tc.tile_pool(name="sp", bufs=2, space="PSUM")),
            "ot": ctx.enter_context(tc.tile_pool(name="ot", bufs=1, space="PSUM")),
        }
        onesr = P["w"].tile([65, 64], BF16, tag="onesr", name="onesr")
        nc.vector.memset(onesr[64:65, :], 1.0)
        P["onesr"] = onesr
        warm = P["w"].tile([1, 1], F32, tag="actwarm", name="actwarm")
        nc.vector.memset(warm[:], 0.0)
        nc.scalar.activation(warm[:], warm[:], EXP)

        # PE p-state warmup: dummy bf16 matmuls on zeroed tiles keep the PE
        # clocking up while the first input DMAs land.
        zw = P["w"].tile([128, 128], BF16, tag="zw", name="zw")
        nc.gpsimd.memset(zw[:], 0.0)
        zw2 = P["w"].tile([128, 512], BF16, tag="zw2", name="zw2")
        nc.gpsimd.memset(zw2[:], 0.0)
        for _ in range(3):
            wps = P["sp"].tile([128, 2, 512], F32, tag="sp", name="wps")
            for r in range(4):
                nc.tensor.matmul(wps[:, 0, :], lhsT=zw[:], rhs=zw2[:],
                                 start=(r == 0), stop=(r == 3),
                                 skip_group_check=True)

        # -------- weights --------
        wts = {}
        for nm in ("wk", "wq", "wv"):
            wts[nm] = nc.dram_tensor(nm, [128, KCH, 128], BF16,
                                     kind="ExternalInput")

        def load_w(nm):
            t = P["w"].tile([128, KCH, 128], BF16, tag=nm, name=nm)
            nc.sync.dma_start(t[:], wts[nm][:])
            wts[nm] = t

        # -------- input staging (issue order == consumption order) --------
        A = phases[0]
        Bp = phases[1] if len(phases) > 1 else None
        for ph in phases:
            for key, chl in (("xq", ph["qch"]), ("xk", ph["kch"]),
                             ("xv", ph["vch"])):
                ph[f"{key}_tiles"] = [None] * len(chl)

        def load_kb(ph):
            s = str(ph["b"])
            kb = P["w"].tile([128, ph["NK"]], F32, tag="kb" + s, name="kb")
            nc.sync.dma_start(kb[:], ph["io"]["kb"][:])
            ph["kb_tile"] = kb

        def stage1(ph, key, ci, eng, halves=1):
            """One input chunk -> SBUF, issued from `eng` (DMA trigger issue
            is ~0.6us+size serial per issuing sequencer, so spread engines)."""
            s = str(ph["b"])
            n = dict(xq=ph["qch"], xk=ph["kch"], xv=ph["vch"])[key][ci][1]
            pool = P["x"] if ph is A else P["xb"]
            xt = pool.tile([128, KCH, n], BF16, tag=f"{key}{s}c{ci}",
                           name=f"{key}{s}c{ci}", bufs=1)
            src = ph["io"][f"{key}c{ci}"]
            step = KCH // halves
            for k in range(0, KCH, step):
                eng.dma_start(xt[:, k:k + step, :], src[:, k:k + step, :])
            ph[f"{key}_tiles"][ci] = xt

        # -------- persistent per-phase tiles --------
        for ph in phases:
            s = str(ph["b"])
            ph["kc"] = P["persist"].tile([128, ph["Kp"]], BF16,
                                         tag="kc" + s, name="kc" + s)
            ph["va"] = P["persist"].tile([128, ph["NK"], 2, 65], BF16,
                                         tag="va" + s, name="va" + s)
            nc.gpsimd.memset(ph["va"][:, :, :, 64:65], 1.0)
            ph["OTs"] = P["persist"].tile([64, 2, ph["Qp"]], BF16,
                                          tag="oT" + s, name="oT" + s)

        # All of phase A's DMA is issued serially from SP in exact
        # consumption order: issue order is the only priority mechanism the
        # 16 shared queues honor, and ring backpressure then throttles SP
        # naturally. Phase B is staged later (inside the chunk-1 emission)
        # on GpSimd software-DGE behind a dependency gate.
        nkA, nqA = len(A["kch"]), len(A["qch"])
        load_w("wk")
        stage1(A, "xk", 0, nc.sync, halves=2)
        load_w("wq")
        stage1(A, "xq", 0, nc.sync)
        load_kb(A)
        if Bp is not None:
            load_kb(Bp)
        load_w("wv")
        stage1(A, "xv", 0, nc.sync)
        for ci in range(1, nkA):
            stage1(A, "xk", ci, nc.sync)
            if ci == nkA - 1 and nqA > 1:
                stage1(A, "xq", 1, nc.sync)
            stage1(A, "xv", ci, nc.sync)
        if nkA == 1 and nqA > 1:
            stage1(A, "xq", 1, nc.sync)
        for ci in range(2, nqA):
            stage1(A, "xq", ci, nc.sync)

        def stage_B():
            # SP's serial trigger FIFO is the throttle: these fire only
            # after all of phase A's transfers have been enqueued
            for ci in range(len(Bp["kch"])):
                stage1(Bp, "xk", ci, nc.sync)
                stage1(Bp, "xv", ci, nc.sync)
            for ci in range(len(Bp["qch"])):
                stage1(Bp, "xq", ci, nc.sync)

        em = _Emitter(nc, P, wts)

        # -------- phase A flow --------
        em.kproj_chunk(A, 0)
        em.qproj_chunk(A, 0)
        rest = deque(_phase_units(em, Bp)) if Bp is not None else deque()
        pending = None
        for ci in range(nqA):
            if ci == 1 and Bp is not None:
                stage_B()
            if ci == 0:
                due = _chunk0_due(em, A)
                anytime = deque()
            else:
                due = deque()
                if pending is not None:
                    due.append((1, pending))
                if ci + 1 < nqA:
                    d = max(3, A["NK"] - 3)
                    st = {}
                    due.append((d - 1, lambda ci=ci, st=st: em.qproj_chunk(
                        A, ci + 1, kr=(0, KCH // 2), st=st)))
                    due.append((d, lambda ci=ci, st=st: em.qproj_chunk(
                        A, ci + 1, kr=(KCH // 2, KCH), st=st)))
                # B's fillers from chunk 2 on (their DMA lands after A's)
                anytime = rest if ci >= 2 else deque()
            otd = em.ladder(A, ci, due, anytime)
            od = em.epilogue_release(A, ci, otd)
            pending = (lambda ci=ci, od=od: em.epilogue_norm(A, ci, od))

        # -------- phase B flow --------
        if Bp is not None:
            if Bp["xk_tiles"][0] is None:
                stage_B()
            while rest:
                rest.popleft()()
            if 0 not in Bp["qcs"]:
                em.qproj_chunk(Bp, 0)
            for ci in range(len(Bp["qch"])):
                due = deque()
                if pending is not None:
                    due.append((1, pending))
                otd = em.ladder(Bp, ci, due, deque())
                if ci + 1 < len(Bp["qch"]):
                    em.qproj_chunk(Bp, ci + 1)
                od = em.epilogue_release(Bp, ci, otd)
                pending = (lambda ci=ci, od=od: em.epilogue_norm(Bp, ci, od))
        if pending is not None:
            pending()

    nc.compile()
    return nc


def _prep_xT(X, Pq):
    """[T, D] -> [128, KCH, Pq] bf16 with x[p, k, t] = X[t, k*128 + p]."""
    Xp = np.ascontiguousarray(X[:Pq].T)                 # [D, Pq]
    return np.ascontiguousarray(
        Xp.reshape(KCH, 128, Pq).transpose(1, 0, 2)).astype(BNP)


def _prep_w(W, c):
    """[D, H*DH] -> per-core [128, KCH, 128] bf16 slice of heads (2c, 2c+1)."""
    Ws = W[:, c * 128:(c + 1) * 128]                    # [D, 128]
    return np.ascontiguousarray(
        Ws.reshape(KCH, 128, 128).transpose(1, 0, 2)).astype(BNP)


def kernel(Q_seq, K_seq, V_seq, Q_len, V_len, WQ, WK, WV):
    global LAST_EXEC_NS
    Q_seq = np.asarray(Q_seq, dtype=np.float32)
    K_seq = np.asarray(K_seq, dtype=np.float32)
    V_seq = np.asarray(V_seq, dtype=np.float32)
    WQ = np.asarray(WQ, dtype=np.float32)
    WK = np.asarray(WK, dtype=np.float32)
    WV = np.asarray(WV, dtype=np.float32)
    qlen = [int(np.asarray(Q_len)[b, 0]) for b in range(B)]
    vlen = [int(np.asarray(V_len)[b, 0]) for b in range(B)]

    phases = []
    for b in range(B):
        Qp = _ceil_div(qlen[b], 32) * 32   # q only needs 32-elem alignment
        if Qp == 0:
            continue  # whole batch output is zero
        if vlen[b] > 0:
            NK, scale = _ceil_div(vlen[b], 128), SCALE
            nfull = vlen[b] // 128      # key tiles with an all-zero bias
        else:
            # all keys masked -> reference softmax degenerates to uniform
            # over all T keys; exp(0*S + 0) = 1 reproduces it exactly.
            NK, scale = T // 128, 0.0
            nfull = NK
        phases.append(dict(b=b, NK=NK, Qp=Qp, Kp=NK * 128, scale=scale,
                           nfull=nfull))
    phases.sort(key=lambda ph: -ph["Qp"])  # big phase first (filler donor)

    out = np.zeros((B, T, H * DH), dtype=np.float32)
    if not phases:
        return out

    nc = _build_program(phases)

    # per-phase data shared by all cores
    shared = {}
    for ph in phases:
        b, s, Qp, Kp, NK = ph["b"], str(ph["b"]), ph["Qp"], ph["Kp"], ph["NK"]
        kbias = np.where(np.arange(Kp) < vlen[b], 0.0,
                         -NEG_BIG if vlen[b] > 0 else 0.0)
        kbias = np.ascontiguousarray(
            kbias.astype(np.float32).reshape(NK, 128).T)        # [128, NK]
        d = {"kb" + s: kbias}
        for key, X, Pq in (("xq", Q_seq[b], Qp), ("xk", K_seq[b], Kp),
                           ("xv", V_seq[b], Kp)):
            full = _prep_xT(X, Pq)                              # [128, KCH, Pq]
            for ci, (c0, n) in enumerate(_chunks(Pq)):
                d[f"{key}{s}c{ci}"] = np.ascontiguousarray(
                    full[:, :, c0:c0 + n])
        shared[s] = d

    in_maps = []
    for c in range(N_CORES):
        m = {}
        for ph in phases:
            m.update(shared[str(ph["b"])])
        m["wq"] = _prep_w(WQ, c)
        m["wk"] = _prep_w(WK, c)
        m["wv"] = _prep_w(WV, c)
        in_maps.append(m)

    trace = bool(os.environ.get("BASS_TRACE"))
    if trace:
        _ensure_ntff_hook()
    res = run_bass_kernel_spmd(nc, in_maps, list(range(N_CORES)), trace=trace)
    LAST_EXEC_NS = res.exec_time_ns

    for c in range(N_CORES):
        r = res.results[c]
        for ph in phases:
            b, s, ql = ph["b"], str(ph["b"]), qlen[ph["b"]]
            o = np.asarray(r["out" + s]).astype(np.float32)  # [64, 2, Qp]
            for h in (0, 1):
                head = 2 * c + h
                out[b, :ql, head * DH:(head + 1) * DH] = o[:, h, :ql].T
    return out


# revision 46
# speedup vs baseline: 1.5582x; 1.0319x over previous
"""Trainium2 Bass kernel: masked multi-head attention, sharded across 8 NeuronCores.

Problem shapes (hardcoded): B=2, T=2048, D=1024, H=16 heads, dh=64.

Sharding: one SPMD program with two phases (one per batch element). In each
phase every core handles 2 of the 16 heads (core c -> heads 2c, 2c+1), so the
16 heads of each batch are spread over all 8 cores. This load-balances the
data-dependent work (Q_len/V_len trim the q/k tile counts per batch).

v2 changes vs the fp32 baseline:
  - bf16 inputs/weights/intermediates: matmuls run at 1 cycle/row instead of
    fp32's 4 (fp32 lowers to 2 half-speed passes on TRN2), DMA bytes halve.
  - The two heads' S^T matmuls (K=64 each) are row-tiled to disjoint PE
    quadrants (tile_position (0,0)/(64,0)) so they execute concurrently.
  - exp() for both heads merged into one ACT instruction over a 2-bank PSUM
    tile [128, 2, n] (ACT is the #2 engine; fewer/larger instrs).
  - Epilogue: numerator copied once (DVE), softmax denominator row pulled out
    of PSUM by a tiny DMA, reciprocal_approx_fast on DVE (the old
    single-lane RECIPROCAL was 2.2us/chunk), broadcast over partitions with a
    K=1 f32r matmul, one fused multiply per head.
  - Query-length masking moved to the host gather (rows >= Q_len are simply
    not copied out; the output buffer is pre-zeroed) - no qmask work on HW.
  - The second batch's projections are emitted as filler units inside the
    first batch's ACT-paced attention ladder to keep the PE busy.
"""

import math
import os
from collections import deque
from contextlib import ExitStack

import numpy as np
import ml_dtypes

import concourse.bacc as bacc
import concourse.mybir as mybir
import concourse.tile as tile
from concourse.bass_utils import run_bass_kernel_spmd

F32 = mybir.dt.float32
F32R = mybir.dt.float32r
BF16 = mybir.dt.bfloat16
EXP = mybir.ActivationFunctionType.Exp
BNP = ml_dtypes.bfloat16

B, T, D, H, DH = 2, 2048, 1024, 16, 64
N_CORES = 8
KCH = D // 128          # 8 contraction chunks of the model dim
NEG_BIG = 1.0e12
SCALE = 1.0 / math.sqrt(DH)

LAST_EXEC_NS = None     # filled when BASS_TRACE=1


def _ensure_ntff_hook():
    """run_bass_kernel_spmd(trace=True) imports antenv.axon_hooks, which some
    containers lack; synthesize it (backed by libaxon_pjrt's NRT profiling)
    so tracing degrades gracefully instead of crashing."""
    import sys
    import types
    try:
        import antenv.axon_hooks  # noqa: F401
        return
    except ImportError:
        pass
    try:
        import antenv
        from trn_agent_boot.trn_boot import _ntff_profile_via_ctypes
        hook = _ntff_profile_via_ctypes("/opt/axon/libaxon_pjrt.so")
    except Exception:
        antenv = None
        hook = None
    try:
        m = types.ModuleType("antenv.axon_hooks")
        m._hook = hook
        m.set_axon_ntff_profile_hook = lambda h: setattr(m, "_hook", h)
        m.get_axon_ntff_profile_hook = lambda: m._hook
        sys.modules["antenv.axon_hooks"] = m
        if antenv is not None:
            antenv.axon_hooks = m
    except Exception:
        pass


def _ceil_div(a, b):
    return -(-a // b)


def _chunks(total, w=512):
    out = []
    c = 0
    while c < total:
        out.append((c, min(w, total - c)))
        c += w
    return out


class _Emitter:
    def __init__(self, nc, P, wts):
        self.nc = nc
        self.P = P
        self.wts = wts

    # ---------- projection units (each returns nothing, emits instrs) ------

    def kproj_chunk(self, ph, ci, kr=(0, KCH), st=None):
        """Project keys chunk ci: kc[:, c0:c0+n] = (WK.T @ K_seq.T) slice.
        `kr` bounds the contraction range so a chunk can be emitted as two
        filler halves sharing the psum tile passed via `st`."""
        nc, P = self.nc, self.P
        c0, n = ph["kch"][ci]
        xt = ph["xk_tiles"][ci]
        if kr[0] == 0:
            ps = P["pp"].tile([128, 512], F32, tag="pp", name="kps")
            if st is not None:
                st["ps"] = ps
        else:
            ps = st["ps"]
        for k in range(*kr):
            nc.tensor.matmul(ps[:, :n], lhsT=self.wts["wk"][:, k, :],
                             rhs=xt[:, k, :n],
                             start=(k == 0), stop=(k == KCH - 1),
                             skip_group_check=True)
        if kr[1] == KCH:
            nc.vector.tensor_copy(ph["kc"][:, c0:c0 + n], ps[:, :n])

    def vproj_tile(self, ph, m):
        """Project value tokens [m*128,(m+1)*128) into va[:, m, :, 0:64]."""
        nc, P = self.nc, self.P
        ci, r = divmod(m * 128, 512)
        c0, cn = ph["vch"][ci]
        xt = ph["xv_tiles"][ci]
        ps = P["pp"].tile([128, 512], F32, tag="pp", name="vps")
        for k in range(KCH):
            nc.tensor.matmul(ps[:, 0:128], lhsT=xt[:, k, r:r + 128],
                             rhs=self.wts["wv"][:, k, :],
                             start=(k == 0), stop=(k == KCH - 1),
                             skip_group_check=True)
        nc.vector.tensor_copy(
            ph["va"][:, m, :, 0:64],
            ps[:, 0:128].rearrange("p (g d) -> p g d", g=2))

    def qproj_chunk(self, ph, ci, kr=(0, KCH), st=None):
        """Project queries chunk ci into the qc ring; returns the tile.
        Like kproj_chunk, can be emitted as two halves via kr/st."""
        nc, P = self.nc, self.P
        c0, n = ph["qch"][ci]
        xt = ph["xq_tiles"][ci]
        if kr[0] == 0:
            ps = P["pp"].tile([128, 512], F32, tag="pp", name="qps")
            if st is not None:
                st["ps"] = ps
        else:
            ps = st["ps"]
        for k in range(*kr):
            nc.tensor.matmul(ps[:, :n], lhsT=self.wts["wq"][:, k, :],
                             rhs=xt[:, k, :n],
                             start=(k == 0), stop=(k == KCH - 1),
                             skip_group_check=True)
        if kr[1] < KCH:
            return None
        qc = P["qc"].tile([128, 512], BF16, tag="qc" + str(ph["b"]),
                          name="qc", bufs=3)
        nc.vector.tensor_copy(qc[:, :n], ps[:, :n])
        ph["qcs"][ci] = qc
        return qc

    # ---------- attention ladder ------------------------------------------

    def ladder(self, ph, ci, due, anytime):
        """S/exp/PV software pipeline for q chunk ci.

        `due`: deque of (deadline_step, closure) in non-decreasing deadline
        order — every unit whose deadline has arrived is emitted that step
        (these carry dataflow deadlines, e.g. vproj(kt) before PV(kt)).
        `anytime`: deque of independent filler closures; at most one is
        popped per step, only on steps with no due unit (keeps PE work per
        step under the ACT exp cadence)."""
        nc, P = self.nc, self.P
        c0, n = ph["qch"][ci]
        NK = ph["NK"]
        qc = ph["qcs"].pop(ci)
        kb = ph["kb_tile"]
        kc, va = ph["kc"], ph["va"]
        scale = ph["scale"]
        nfull = ph["nfull"]    # leading key tiles with all-zero bias

        otd = P["ot"].tile([65, 2, 512], F32, tag="ot", name="otd")

        # group key tiles: zero-bias tiles share one sps tile + ONE exp
        # (per-instruction ACT overhead dominates for narrow chunks)
        cap = max(1, 512 // n)
        groups, kt = [], 0
        while kt < NK:
            g = min(nfull - kt, cap) if kt < nfull else 1
            g = max(g, 1)
            groups.append((kt, g))
            kt += g

        def emit_sg(gi):
            kt0, g = groups[gi]
            sps = P["sp"].tile([128, 2, 512], F32, tag="sp", name="sps")
            for j in range(g):
                for h in (0, 1):
                    kt = kt0 + j
                    nc.tensor.matmul(
                        sps[:, h, j * n:(j + 1) * n],
                        lhsT=kc[h * 64:(h + 1) * 64,
                                kt * 128:(kt + 1) * 128],
                        rhs=qc[h * 64:(h + 1) * 64, :n],
                        start=True, stop=True,
                        tile_position=(h * 64, 0),
                        skip_group_check=True)
            e = P["e"].tile([128, 2, 512], BF16, tag="e", name="e", bufs=4)
            bias = 0.0 if kt0 + g <= nfull else kb[:, kt0:kt0 + 1]
            nc.scalar.activation(e[:, :, :g * n], sps[:, :, :g * n], EXP,
                                 bias=bias, scale=scale)
            return e

        ep = emit_sg(0)
        for gi in range(len(groups)):
            ec = ep
            if gi + 1 < len(groups):
                ep = emit_sg(gi + 1)
            kt0, g = groups[gi]
            popped = 0
            while due and due[0][0] <= kt0 + g - 1:
                due.popleft()[1]()
                popped += 1
            while popped < g and anytime:
                anytime.popleft()()
                popped += 1
            for j in range(g):
                kt = kt0 + j
                for h in (0, 1):
                    nc.tensor.matmul(otd[:, h, :n], lhsT=va[:, kt, h, :],
                                     rhs=ec[:, h, j * n:(j + 1) * n],
                                     start=(kt == 0), stop=(kt == NK - 1),
                                     skip_group_check=True)
        return otd

    def epilogue_release(self, ph, ci, otd):
        """Copy numerator + denominator row out of PSUM (frees otd fast)."""
        nc, P = self.nc, self.P
        c0, n = ph["qch"][ci]
        od = P["ou"].tile([65, 2, 512], BF16, tag="od", name="od", bufs=2)
        nc.vector.tensor_copy(od[:, :, :n], otd[:, :, :n])
        return od

    def epilogue_norm(self, ph, ci, od):
        """Normalize -> OTs slice and DMA it out (no qmask: host trims).
        Deferred into the next ladder via a due-unit so its PE/DVE work
        doesn't block the next chunk's S matmuls in the engine FIFOs."""
        nc, P = self.nc, self.P
        c0, n = ph["qch"][ci]
        rsb = P["rows"].tile([64, 2, 512], F32, tag="rsb", name="rsb",
                             bufs=2)
        for h in (0, 1):
            # broadcast d over 64 partitions (K=1 bf16 matmul), then
            # reciprocal on the [64, n] block (DVE cost is free-size-based,
            # so this is no dearer than a single-partition reciprocal).
            dps = P["pp"].tile([128, 512], F32, tag="pp", name="dps")
            nc.tensor.matmul(dps[0:64, :n],
                             lhsT=P["onesr"][64:65, 0:64],
                             rhs=od[64:65, h, :n],
                             start=True, stop=True, skip_group_check=True)
            nc.vector.reciprocal_approx_fast(rsb[:, h, :n], dps[0:64, :n])
            nc.vector.tensor_mul(ph["OTs"][:, h, c0:c0 + n],
                                 od[0:64, h, :n], rsb[:, h, :n])
        nc.gpsimd.dma_start(ph["io"]["out"][:, :, c0:c0 + n],
                            ph["OTs"][:, :, c0:c0 + n])


def _chunk0_due(em, ph):
    """Deadline units for the first q chunk's ladder: the phase's remaining
    k-proj chunks and all v-proj tiles, interleaved in the exact order their
    DMA chunks arrive, plus qproj(1). Deadlines: vproj(m) before PV(m)
    (popped a step early so the DVE copy hides), kproj(ci) before S(4ci)
    which is emitted at step 4ci-1, qproj(1) a few steps before chunk end."""
    NK = ph["NK"]
    due = []
    for m in range(NK):
        if m >= 1 and m % 4 == 0:
            due.append((max(0, m - 3),
                        lambda ci=m // 4: em.kproj_chunk(ph, ci)))
        due.append((max(0, m - 1), lambda m=m: em.vproj_tile(ph, m)))
    if len(ph["qch"]) > 1:
        due.append((max(0, NK - 3), lambda: em.qproj_chunk(ph, 1)))
    due.sort(key=lambda u: u[0])   # stable: ties keep DMA-arrival order
    return deque(due)


def _phase_units(em, ph):
    """Independent filler closures projecting all of phase `ph`'s inputs,
    in DMA-arrival order. kproj chunks are split in two halves so a single
    pop stays under the ladder's per-step PE budget."""
    units = []
    for ci in range(len(ph["kch"])):
        st = {}
        units.append(lambda ci=ci, st=st:
                     em.kproj_chunk(ph, ci, kr=(0, KCH // 2), st=st))
        units.append(lambda ci=ci, st=st:
                     em.kproj_chunk(ph, ci, kr=(KCH // 2, KCH), st=st))
        for m in range(ci * 4, min((ci + 1) * 4, ph["NK"])):
            units.append(lambda m=m: em.vproj_tile(ph, m))
    units.append(lambda: em.qproj_chunk(ph, 0))
    return units


def _build_program(phases):
    nc = bacc.Bacc("TRN2", target_bir_lowering=False, debug=False,
                   num_devices=N_CORES)
    for ph in phases:
        s = str(ph["b"])
        Qp, Kp, NK = ph["Qp"], ph["Kp"], ph["NK"]
        ph["qch"] = _chunks(Qp)
        ph["kch"] = _chunks(Kp)
        ph["vch"] = ph["kch"]
        ph["qcs"] = {}
        io = {
            "kb": nc.dram_tensor("kb" + s, [128, NK], F32, kind="ExternalInput"),
            "out": nc.dram_tensor("out" + s, [64, 2, Qp], BF16, kind="ExternalOutput"),
        }
        # per-chunk input tensors: per-partition-contiguous so each DMA
        # lowers to 128 large descriptors instead of 1KB-strided fragments
        for key, chl in (("xq", ph["qch"]), ("xk", ph["kch"]), ("xv", ph["vch"])):
            for ci, (c0, n) in enumerate(chl):
                io[f"{key}c{ci}"] = nc.dram_tensor(
                    f"{key}{s}c{ci}", [128, KCH, n], BF16, kind="ExternalInput")
        ph["io"] = io

    with tile.TileContext(nc) as tc, ExitStack() as ctx:
        P = {
            "w": ctx.enter_context(tc.tile_pool(name="w", bufs=1)),
            "x": ctx.enter_context(tc.tile_pool(name="x", bufs=1)),
            "xb": ctx.enter_context(tc.tile_pool(name="xb", bufs=1)),
            "qc": ctx.enter_context(tc.tile_pool(name="qc", bufs=3)),
            "e": ctx.enter_context(tc.tile_pool(name="e", bufs=4)),
            "ou": ctx.enter_context(tc.tile_pool(name="ou", bufs=2)),
            "rows": ctx.enter_context(tc.tile_pool(name="rows", bufs=2)),
            "persist": ctx.enter_context(tc.tile_pool(name="persist", bufs=1)),
            "pp": ctx.enter_context(tc.tile_pool(name="pp", bufs=2, space="PSUM")),
            "sp": ctx.enter_context(tc.tile_pool(name="sp", bufs=2, space="PSUM")),
            "ot": ctx.enter_context(tc.tile_pool(name="ot", bufs=1, space="PSUM")),
        }
        onesr = P["w"].tile([65, 64], BF16, tag="onesr", name="onesr")
        nc.vector.memset(onesr[64:65, :], 1.0)
        P["onesr"] = onesr
        warm = P["w"].tile([1, 1], F32, tag="actwarm", name="actwarm")
        nc.vector.memset(warm[:], 0.0)
        nc.scalar.activation(warm[:], warm[:], EXP)

        # PE p-state warmup: dummy bf16 matmuls on zeroed tiles keep the PE
        # clocking up while the first input DMAs land.
        zw = P["w"].tile([128, 128], BF16, tag="zw", name="zw")
        nc.gpsimd.memset(zw[:], 0.0)
        zw2 = P["w"].tile([128, 512], BF16, tag="zw2", name="zw2")
        nc.gpsimd.memset(zw2[:], 0.0)
        for _ in range(3):
            wps = P["sp"].tile([128, 2, 512], F32, tag="sp", name="wps")
            for r in range(4):
                nc.tensor.matmul(wps[:, 0, :], lhsT=zw[:], rhs=zw2[:],
                                 start=(r == 0), stop=(r == 3),
                                 skip_group_check=True)

        # -------- weights --------
        wts = {}
        for nm in ("wk", "wq", "wv"):
            wts[nm] = nc.dram_tensor(nm, [128, KCH, 128], BF16,
                                     kind="ExternalInput")

        def load_w(nm):
            t = P["w"].tile([128, KCH, 128], BF16, tag=nm, name=nm)
            nc.sync.dma_start(t[:], wts[nm][:])
            wts[nm] = t

        # -------- input staging (issue order == consumption order) --------
        A = phases[0]
        Bp = phases[1] if len(phases) > 1 else None
        for ph in phases:
            for key, chl in (("xq", ph["qch"]), ("xk", ph["kch"]),
                             ("xv", ph["vch"])):
                ph[f"{key}_tiles"] = [None] * len(chl)

        def load_kb(ph):
            s = str(ph["b"])
            kb = P["w"].tile([128, ph["NK"]], F32, tag="kb" + s, name="kb")
            nc.sync.dma_start(kb[:], ph["io"]["kb"][:])
            ph["kb_tile"] = kb

        def stage1(ph, key, ci, eng, halves=1):
            """One input chunk -> SBUF, issued from `eng` (DMA trigger issue
            is ~0.6us+size serial per issuing sequencer, so spread engines)."""
            s = str(ph["b"])
            n = dict(xq=ph["qch"], xk=ph["kch"], xv=ph["vch"])[key][ci][1]
            pool = P["x"] if ph is A else P["xb"]
            xt = pool.tile([128, KCH, n], BF16, tag=f"{key}{s}c{ci}",
                           name=f"{key}{s}c{ci}", bufs=1)
            src = ph["io"][f"{key}c{ci}"]
            step = KCH // halves
            for k in range(0, KCH, step):
                eng.dma_start(xt[:, k:k + step, :], src[:, k:k + step, :])
            ph[f"{key}_tiles"][ci] = xt

        # -------- persistent per-phase tiles --------
        for ph in phases:
            s = str(ph["b"])
            ph["kc"] = P["persist"].tile([128, ph["Kp"]], BF16,
                                         tag="kc" + s, name="kc" + s)
            ph["va"] = P["persist"].tile([128, ph["NK"], 2, 65], BF16,
                                         tag="va" + s, name="va" + s)
            nc.gpsimd.memset(ph["va"][:, :, :, 64:65], 1.0)
            ph["OTs"] = P["persist"].tile([64, 2, ph["Qp"]], BF16,
                                          tag="oT" + s, name="oT" + s)

        # All of phase A's DMA is issued serially from SP in exact
        # consumption order: issue order is the only priority mechanism the
        # 16 shared queues honor, and ring backpressure then throttles SP
        # naturally. Phase B is staged later (inside the chunk-1 emission)
        # on GpSimd software-DGE behind a dependency gate.
        nkA, nqA = len(A["kch"]), len(A["qch"])
        load_w("wk")
        stage1(A, "xk", 0, nc.sync, halves=2)
        load_w("wq")
        stage1(A, "xq", 0, nc.sync, halves=2)
        load_kb(A)
        if Bp is not None:
            load_kb(Bp)
        load_w("wv")
        stage1(A, "xv", 0, nc.sync)
        for ci in range(1, nkA):
            stage1(A, "xk", ci, nc.sync)
            if ci == nkA - 1 and nqA > 1:
                stage1(A, "xq", 1, nc.sync)
            stage1(A, "xv", ci, nc.sync)
        if nkA == 1 and nqA > 1:
            stage1(A, "xq", 1, nc.sync)
        for ci in range(2, nqA):
            stage1(A, "xq", ci, nc.sync)

        def stage_B():
            # SP's serial trigger FIFO is the throttle: these fire only
            # after all of phase A's transfers have been enqueued
            for ci in range(len(Bp["kch"])):
                stage1(Bp, "xk", ci, nc.sync)
                stage1(Bp, "xv", ci, nc.sync)
            for ci in range(len(Bp["qch"])):
                stage1(Bp, "xq", ci, nc.sync)

        em = _Emitter(nc, P, wts)

        # -------- phase A flow (head kproj/qproj in halves so each half
        # starts as soon as its half-tile DMA lands) --------
        stk, stq = {}, {}
        em.kproj_chunk(A, 0, kr=(0, KCH // 2), st=stk)
        em.kproj_chunk(A, 0, kr=(KCH // 2, KCH), st=stk)
        em.qproj_chunk(A, 0, kr=(0, KCH // 2), st=stq)
        em.qproj_chunk(A, 0, kr=(KCH // 2, KCH), st=stq)
        rest = deque(_phase_units(em, Bp)) if Bp is not None else deque()
        pending = None
        for ci in range(nqA):
            if ci == 1 and Bp is not None:
                stage_B()
            if ci == 0:
                due = _chunk0_due(em, A)
                anytime = deque()
            else:
                due = deque()
                if pending is not None:
                    due.append((1, pending))
                if ci + 1 < nqA:
                    d = max(3, A["NK"] - 3)
                    st = {}
                    due.append((d - 1, lambda ci=ci, st=st: em.qproj_chunk(
                        A, ci + 1, kr=(0, KCH // 2), st=st)))
                    due.append((d, lambda ci=ci, st=st: em.qproj_chunk(
                        A, ci + 1, kr=(KCH // 2, KCH), st=st)))
                # B's fillers once its DMA has landed (throttled by SP order)
                anytime = rest if ci >= 1 else deque()
            otd = em.ladder(A, ci, due, anytime)
            od = em.epilogue_release(A, ci, otd)
            pending = (lambda ci=ci, od=od: em.epilogue_norm(A, ci, od))

        # -------- phase B flow --------
        if Bp is not None:
            if Bp["xk_tiles"][0] is None:
                stage_B()
            while rest:
                rest.popleft()()
            if 0 not in Bp["qcs"]:
                em.qproj_chunk(Bp, 0)
            for ci in range(len(Bp["qch"])):
                due = deque()
                if pending is not None:
                    due.append((1, pending))
                otd = em.ladder(Bp, ci, due, deque())
                if ci + 1 < len(Bp["qch"]):
                    em.qproj_chunk(Bp, ci + 1)
                od = em.epilogue_release(Bp, ci, otd)
                pending = (lambda ci=ci, od=od: em.epilogue_norm(Bp, ci, od))
        if pending is not None:
            pending()

    nc.compile()
    return nc


def _prep_xT(X, Pq):
    """[T, D] -> [128, KCH, Pq] bf16 with x[p, k, t] = X[t, k*128 + p]."""
    Xp = np.ascontiguousarray(X[:Pq].T)                 # [D, Pq]
    return np.ascontiguousarray(
        Xp.reshape(KCH, 128, Pq).transpose(1, 0, 2)).astype(BNP)


def _prep_w(W, c):
    """[D, H*DH] -> per-core [128, KCH, 128] bf16 slice of heads (2c, 2c+1)."""
    Ws = W[:, c * 128:(c + 1) * 128]                    # [D, 128]
    return np.ascontiguousarray(
        Ws.reshape(KCH, 128, 128).transpose(1, 0, 2)).astype(BNP)


def kernel(Q_seq, K_seq, V_seq, Q_len, V_len, WQ, WK, WV):
    global LAST_EXEC_NS
    Q_seq = np.asarray(Q_seq, dtype=np.float32)
    K_seq = np.asarray(K_seq, dtype=np.float32)
    V_seq = np.asarray(V_seq, dtype=np.float32)
    WQ = np.asarray(WQ, dtype=np.float32)
    WK = np.asarray(WK, dtype=np.float32)
    WV = np.asarray(WV, dtype=np.float32)
    qlen = [int(np.asarray(Q_len)[b, 0]) for b in range(B)]
    vlen = [int(np.asarray(V_len)[b, 0]) for b in range(B)]

    phases = []
    for b in range(B):
        Qp = _ceil_div(qlen[b], 32) * 32   # q only needs 32-elem alignment
        if Qp == 0:
            continue  # whole batch output is zero
        if vlen[b] > 0:
            NK, scale = _ceil_div(vlen[b], 128), SCALE
            nfull = vlen[b] // 128      # key tiles with an all-zero bias
        else:
            # all keys masked -> reference softmax degenerates to uniform
            # over all T keys; exp(0*S + 0) = 1 reproduces it exactly.
            NK, scale = T // 128, 0.0
            nfull = NK
        phases.append(dict(b=b, NK=NK, Qp=Qp, Kp=NK * 128, scale=scale,
                           nfull=nfull))
    phases.sort(key=lambda ph: -ph["Qp"])  # big phase first (filler donor)

    out = np.zeros((B, T, H * DH), dtype=np.float32)
    if not phases:
        return out

    nc = _build_program(phases)

    # per-phase data shared by all cores
    shared = {}
    for ph in phases:
        b, s, Qp, Kp, NK = ph["b"], str(ph["b"]), ph["Qp"], ph["Kp"], ph["NK"]
        kbias = np.where(np.arange(Kp) < vlen[b], 0.0,
                         -NEG_BIG if vlen[b] > 0 else 0.0)
        kbias = np.ascontiguousarray(
            kbias.astype(np.float32).reshape(NK, 128).T)        # [128, NK]
        d = {"kb" + s: kbias}
        for key, X, Pq in (("xq", Q_seq[b], Qp), ("xk", K_seq[b], Kp),
                           ("xv", V_seq[b], Kp)):
            full = _prep_xT(X, Pq)                              # [128, KCH, Pq]
            for ci, (c0, n) in enumerate(_chunks(Pq)):
                d[f"{key}{s}c{ci}"] = np.ascontiguousarray(
                    full[:, :, c0:c0 + n])
        shared[s] = d

    in_maps = []
    for c in range(N_CORES):
        m = {}
        for ph in phases:
            m.update(shared[str(ph["b"])])
        m["wq"] = _prep_w(WQ, c)
        m["wk"] = _prep_w(WK, c)
        m["wv"] = _prep_w(WV, c)
        in_maps.append(m)

    trace = bool(os.environ.get("BASS_TRACE"))
    if trace:
        _ensure_ntff_hook()
    res = run_bass_kernel_spmd(nc, in_maps, list(range(N_CORES)), trace=trace)
    LAST_EXEC_NS = res.exec_time_ns

    for c in range(N_CORES):
        r = res.results[c]
        for ph in phases:
            b, s, ql = ph["b"], str(ph["b"]), qlen[ph["b"]]
            o = np.asarray(r["out" + s]).astype(np.float32)  # [64, 2, Qp]
            for h in (0, 1):
                head = 2 * c + h
                out[b, :ql, head * DH:(head + 1) * DH] = o[:, h, :ql].T
    return out
